# revision 17
# baseline (speedup 1.0000x reference)
"""DeepSeek-V2-Lite MoE layer on 8 Trainium2 NeuronCores.

Strategy (expert-parallel, per the sharding hint):
  - Host computes the gate (256x64 matmul + softmax + top-6) in fp32 numpy --
    this is the token dispatch decision, which necessarily lives on the host
    since the host builds the per-core input shards ("all-to-all" realized as
    host-side gather/scatter under the full-IO contract).
  - Each core owns 8 routed experts (weights sharded on the expert axis) and
    a 1/8 slice of the shared expert intermediate dim (tensor-parallel).
  - Tokens routed to each expert are gathered host-side into a fixed-capacity
    [C] batch (C = max expert load rounded up; uniform so the single SPMD
    program is identical across cores).
  - On device, all matmuls are token-stationary: the (small) token batch is
    the stationary PE operand, the expert weights stream through as the
    moving operand, so PE time ~ weight-columns/2.4GHz and the kernel is
    bound by the irreducible weight DMA (~138 MB/core in bf16).
  - Device applies the per-token routing weight; host combine is a pure
    gather-sum plus the 8-way shared-expert partial sum.
"""

import os
import numpy as np
import ml_dtypes

BF16 = ml_dtypes.bfloat16
F8E3 = ml_dtypes.float8_e3m4   # TRN FP8_EXP3: 4 mantissa bits, max 15.5

HIDDEN = 2048
FFN = 1408
N_EXPERTS = 64
TOP_K = 6
SHARED_FFN = 2816          # 2 shared experts * FFN
T = 256
N_CORES = 8
EPC = N_EXPERTS // N_CORES  # experts per core = 8
SFS = SHARED_FFN // N_CORES  # shared-FFN slice per core = 352

# routed expert weights are shipped in fp8-e3m4 scaled by WSCALE (so the
# +/-0.1-ish gaussian weights land in e3m4's normal range); the gate-side
# 1/WSCALE is undone inside the silu activation, the remaining WSCALE^2
# factor is folded into the per-token combine weights.
WSCALE = 64.0

# gate/up column interleave: stream order [g0|u0|g1|u1|g2|u2], pair widths
PAIR_W = [512, 512, 384]
PAIR_OFF = [0, 1024, 2048]          # start col of each pair block (2*w wide)
N_KH = HIDDEN // 128                # 16 K-chunks over hidden
N_KF = FFN // 128                   # 11 K-chunks over FFN

_PROGRAM_CACHE = {}
LAST_RESULTS = None


def _route(x, gate_w):
    """fp32 softmax top-k routing, matching jax.lax.top_k tie-breaking
    (stable sort -> lowest index wins ties)."""
    logits = x @ gate_w.T                      # [T, E] fp32
    m = logits.max(axis=-1, keepdims=True)
    e = np.exp(logits - m)
    scores = e / e.sum(axis=-1, keepdims=True)
    ids = np.argsort(-scores, axis=-1, kind="stable")[:, :TOP_K]
    w = np.take_along_axis(scores, ids, axis=-1)
    w = w / (w.sum(axis=-1, keepdims=True) + 1e-20)
    return ids, w.astype(np.float32)


def _build_program(C):
    import concourse.bass as bass
    import concourse.bacc as bacc
    import concourse.mybir as mybir
    import concourse.tile as tile
    from concourse.masks import make_identity
    from contextlib import ExitStack

    f32 = mybir.dt.float32
    bf16 = mybir.dt.bfloat16
    SILU = mybir.ActivationFunctionType.Silu

    # Bacc (not plain Bass): its compile pipeline splits multi-wait
    # instructions into the 1-wait-per-instruction form TRN2 requires.
    nc = bacc.Bacc(None)

    f8 = mybir.dt.float8e3

    # DRAM layouts are host-prepped into final SBUF layouts so every weight
    # DMA is contiguous per partition row.
    W_GU = N_KH * 2816
    W_DN = 2 * N_KF * 1024
    d_xt = nc.dram_tensor("xt", [EPC, 128, N_KH * C], bf16, kind="ExternalInput")
    d_wgu = nc.dram_tensor("wgu", [EPC, 128, W_GU], f8, kind="ExternalInput")
    d_wdn = nc.dram_tensor("wdn", [EPC, 128, W_DN], f8, kind="ExternalInput")
    d_wv = nc.dram_tensor("wv", [C, EPC], f32, kind="ExternalInput")
    d_xsh = nc.dram_tensor("xsh", [128, N_KH * 256], bf16, kind="ExternalInput")
    d_wsgu = nc.dram_tensor("wsgu", [128, N_KH * 2 * SFS], bf16, kind="ExternalInput")
    d_wsd = nc.dram_tensor("wsd", [128, 3 * 2048], bf16, kind="ExternalInput")
    d_yrt = nc.dram_tensor("yrt", [EPC, C, HIDDEN], bf16, kind="ExternalOutput")
    d_ysh = nc.dram_tensor("ysh", [T, HIDDEN], bf16, kind="ExternalOutput")

    with tile.TileContext(nc) as tc, ExitStack() as ctx:
        p_const = ctx.enter_context(tc.tile_pool(name="const", bufs=1))
        p_wgu = ctx.enter_context(tc.tile_pool(name="wgu", bufs=3))
        p_wdn = ctx.enter_context(tc.tile_pool(name="wdn", bufs=4))
        p_wsh = ctx.enter_context(tc.tile_pool(name="wsh", bufs=1))
        p_xt = ctx.enter_context(tc.tile_pool(name="xt", bufs=2))
        p_act = ctx.enter_context(tc.tile_pool(name="act", bufs=2))
        p_gs = ctx.enter_context(tc.tile_pool(name="gs", bufs=2))
        p_actT = ctx.enter_context(tc.tile_pool(name="actT", bufs=2))
        p_out = ctx.enter_context(tc.tile_pool(name="out", bufs=2))
        ps_gu = ctx.enter_context(tc.tile_pool(name="ps_gu", bufs=2, space="PSUM"))
        ps_y = ctx.enter_context(tc.tile_pool(name="ps_y", bufs=1, space="PSUM"))
        ps_t = ctx.enter_context(tc.tile_pool(name="ps_t", bufs=2, space="PSUM"))

        ident = p_const.tile([128, 128], bf16)
        make_identity(nc, ident[:])
        wv_t = p_const.tile([C, EPC], f32)
        nc.sync.dma_start(out=wv_t[:], in_=d_wv[:])

        state = {}

        def routed_gu(s):
            """gate+up matmuls and silu*u for expert slot s"""
            xt = p_xt.tile([128, N_KH * C], bf16, tag="xt")
            nc.sync.dma_start(out=xt[:], in_=d_xt[s])
            act = p_act.tile([C, FFN], bf16, tag="act")
            # ---- gate+up, pair-major over 3 (g,u) column pairs ----
            for pr in range(3):
                w = PAIR_W[pr]
                if s == 0 and pr == 0:
                    # first weights: one tile per pair of k-chunks so the PE
                    # starts as soon as the first 256KB lands (tile-granular
                    # dependency tracking would otherwise wait for all 2MB)
                    wgs = []
                    for q in range(8):
                        wq = p_wgu.tile([128, 2 * 2 * w], f8, tag=f"wgu0_{q}",
                                        bufs=1)
                        nc.sync.dma_start(
                            out=wq[:],
                            in_=d_wgu[s, :, q * 4 * w:(q + 1) * 4 * w],
                        )
                        wgs.append(wq)
                    wgof = lambda k: (wgs[k // 2], (k % 2) * 2 * w)
                else:
                    wg = p_wgu.tile([128, N_KH * 2 * w], f8, tag="wgu")
                    nc.sync.dma_start(
                        out=wg[:],
                        in_=d_wgu[s, :, N_KH * PAIR_OFF[pr]:
                                 N_KH * (PAIR_OFF[pr] + 2 * w)],
                    )
                    wgof = lambda k: (wg, k * 2 * w)
                pg = ps_gu.tile([C, 1024], mybir.dt.float32, tag="pg")
                for k in range(N_KH):
                    lhs = xt[:, k * C:(k + 1) * C]
                    wt, o = wgof(k)
                    nc.tensor.matmul(
                        pg[:, 0:w], lhs, wt[:, o:o + w],
                        start=(k == 0), stop=(k == N_KH - 1),
                    )
                    nc.tensor.matmul(
                        pg[:, 512:512 + w], lhs, wt[:, o + w:o + 2 * w],
                        start=(k == 0), stop=(k == N_KH - 1),
                    )
                gs = p_gs.tile([C, 512], mybir.dt.float32, tag="gs")
                # silu(G~ / WSCALE) undoes the gate-side weight scale exactly
                nc.scalar.activation(gs[:, :w], pg[:, :w], SILU, scale=1.0 / WSCALE)
                nc.vector.tensor_mul(
                    act[:, pr * 512: pr * 512 + w], gs[:, :w], pg[:, 512:512 + w]
                )
            # prefetch down weights now so they stream during the next gu block
            wds = []
            for h in range(2):
                wd = p_wdn.tile([128, N_KF * 1024], f8, tag="wdn")
                nc.sync.dma_start(
                    out=wd[:],
                    in_=d_wdn[s, :, h * N_KF * 1024:(h + 1) * N_KF * 1024],
                )
                wds.append(wd)
            state[s] = (act, wds)

        def routed_tail(s):
            """transpose + down projection + drain for expert slot s"""
            act, wds = state.pop(s)
            # ---- transpose act (tokens->free) for the down matmul ----
            # chunk transposes land in grouped PSUM tiles; 3 groups so the
            # first group's DVE copy drains while the PE transposes the rest
            # (the first down matmul then never waits on the copy latency)
            actT = p_actT.tile([128, N_KF * C], bf16, tag="actT")
            TG = (N_KF + 2) // 3
            j = 0
            while j < N_KF:
                g = min(TG, N_KF - j)
                pt = ps_t.tile([128, 512], bf16, tag="pt")
                for i in range(g):
                    nc.tensor.transpose(
                        pt[:, i * C:(i + 1) * C],
                        act[:, (j + i) * 128:(j + i + 1) * 128], ident[:C, :C]
                    )
                nc.vector.tensor_copy(actT[:, j * C:(j + g) * C], pt[:, :g * C])
                j += g
            # ---- down projection, N-half major ----
            out_sb = p_out.tile([C, HIDDEN], bf16, tag="out")
            for h in range(2):
                py = ps_y.tile([C, 1024], mybir.dt.float32, tag="py")
                for k in range(N_KF):
                    for n in range(2):
                        nc.tensor.matmul(
                            py[:, n * 512:(n + 1) * 512],
                            actT[:, k * C:(k + 1) * C],
                            wds[h][:, k * 1024 + n * 512: k * 1024 + (n + 1) * 512],
                            start=(k == 0), stop=(k == N_KF - 1),
                        )
                # routed combine weight folded in during PSUM drain
                nc.vector.tensor_scalar_mul(
                    out_sb[:, h * 1024:(h + 1) * 1024], py[:], wv_t[:, s:s + 1]
                )
            nc.sync.dma_start(out=d_yrt[s], in_=out_sb[:])

        def shared_expert():
            xsh = p_xt.tile([128, N_KH * 256], bf16, tag="xsh")
            nc.sync.dma_start(out=xsh[:], in_=d_xsh[:])
            wsg = p_wsh.tile([128, N_KH * 2 * SFS], bf16, tag="wsg")
            nc.sync.dma_start(out=wsg[:], in_=d_wsgu[:])
            wsd = p_wsh.tile([128, 3 * 2048], bf16, tag="wsd")
            nc.sync.dma_start(out=wsd[:], in_=d_wsd[:])
            for g in range(2):  # two groups of 128 tokens
                pg = ps_gu.tile([128, 1024], mybir.dt.float32, tag="pg")
                for k in range(N_KH):
                    lhs = xsh[:, k * 256 + g * 128: k * 256 + g * 128 + 128]
                    nc.tensor.matmul(
                        pg[:, 0:SFS], lhs, wsg[:, k * 2 * SFS: k * 2 * SFS + SFS],
                        start=(k == 0), stop=(k == N_KH - 1),
                    )
                    nc.tensor.matmul(
                        pg[:, 512:512 + SFS],
                        lhs, wsg[:, k * 2 * SFS + SFS: (k + 1) * 2 * SFS],
                        start=(k == 0), stop=(k == N_KH - 1),
                    )
                gs = p_gs.tile([128, 512], mybir.dt.float32, tag="gs")
                nc.scalar.activation(gs[:, :SFS], pg[:, :SFS], SILU)
                act_sh = p_act.tile([128, SFS], bf16, tag="act")
                nc.vector.tensor_mul(act_sh[:], gs[:, :SFS], pg[:, 512:512 + SFS])
                actT_sh = p_actT.tile([128, 3 * 128], bf16, tag="actT")
                # rows 96:128 of the last K-chunk pair with zero weight rows;
                # zero them so junk*0 can't produce NaN
                nc.vector.memset(actT_sh[:], 0.0)
                for j, wj in enumerate([128, 128, 96]):
                    pt = ps_t.tile([128, 128], bf16, tag="pt")
                    nc.tensor.transpose(
                        pt[:wj, :], act_sh[:, j * 128: j * 128 + wj], ident[:, :]
                    )
                    nc.vector.tensor_copy(
                        actT_sh[:wj, j * 128:(j + 1) * 128], pt[:wj, :]
                    )
                out_sh = p_out.tile([128, HIDDEN], bf16, tag="out")
                for h in range(2):
                    py = ps_y.tile([128, 1024], mybir.dt.float32, tag="py")
                    for k in range(3):
                        for n in range(2):
                            nc.tensor.matmul(
                                py[:, n * 512:(n + 1) * 512],
                                actT_sh[:, k * 128:(k + 1) * 128],
                                wsd[:, k * 2048 + h * 1024 + n * 512:
                                    k * 2048 + h * 1024 + (n + 1) * 512],
                                start=(k == 0), stop=(k == 2),
                            )
                    nc.vector.tensor_copy(out_sh[:, h * 1024:(h + 1) * 1024], py[:])
                nc.sync.dma_start(out=d_ysh[g * 128:(g + 1) * 128, :], in_=out_sh[:])

        # one-expert software-pipeline skew: the silu->mul->transpose chain of
        # expert s hides under expert s+1's gate/up matmuls; the shared expert
        # fills the same latency for the last slot and keeps the kernel tail
        # down to a small routed-output drain
        routed_gu(0)
        for s in range(1, EPC):
            routed_gu(s)
            routed_tail(s - 1)
        shared_expert()
        routed_tail(EPC - 1)

    if not nc.is_finalized():
        nc.finalize()
    return nc


def _sbufize(a, kdim):
    """[K*128, N] -> [128, K*N] SBUF layout (K-chunks along free dim)."""
    K = a.shape[0] // 128
    return np.ascontiguousarray(
        a.reshape(K, 128, -1).transpose(1, 0, 2).reshape(128, -1)
    )


def kernel(hidden_states, gate_w, w_gate_up, w_down, ws_gate_up, ws_down):
    global LAST_RESULTS
    x = np.asarray(hidden_states, dtype=np.float32).reshape(T, HIDDEN)
    gate_w = np.asarray(gate_w, dtype=np.float32)

    ids, tw = _route(x, gate_w)

    # per-expert token lists + positions
    lists = [[] for _ in range(N_EXPERTS)]
    pos = np.zeros((T, TOP_K), dtype=np.int64)
    for t in range(T):
        for i in range(TOP_K):
            e = ids[t, i]
            pos[t, i] = len(lists[e])
            lists[e].append(t)
    maxload = max(len(l) for l in lists)
    C = max(32, -(-maxload // 16) * 16)
    assert C <= 128, f"expert overload {maxload}: splitting not implemented"

    xb = x.astype(BF16)
    xT = np.ascontiguousarray(x.T)  # fp32 [H, T]

    # column permutation interleaving gate/up rows into [g0|u0|g1|u1|g2|u2]
    perm = np.concatenate([
        np.concatenate([np.arange(o, o + w), FFN + np.arange(o, o + w)])
        for o, w in zip([0, 512, 1024], PAIR_W)
    ])

    w_gate_up = np.asarray(w_gate_up)
    w_down = np.asarray(w_down)
    ws_gate_up = np.asarray(ws_gate_up)
    ws_down = np.asarray(ws_down)

    def _q8(a):
        return np.clip(a * WSCALE, -15.5, 15.5).astype(F8E3)

    in_maps = []
    for c in range(N_CORES):
        # routed expert weights (fp8-e3m4, scaled), token batches (bf16)
        wgu = np.empty((EPC, 128, N_KH * 2816), dtype=F8E3)
        wdn = np.empty((EPC, 128, 2 * N_KF * 1024), dtype=F8E3)
        xts = np.zeros((EPC, 128, N_KH * C), dtype=BF16)
        wv = np.zeros((C, EPC), dtype=np.float32)
        for s in range(EPC):
            e = c * EPC + s
            wt = _q8(w_gate_up[e][perm].T)              # [H, 2816] interleaved
            off = 0
            for o, w in zip(PAIR_OFF, PAIR_W):
                blk = _sbufize(wt[:, o:o + 2 * w], N_KH)  # [128, 16*2w]
                wgu[s, :, off:off + blk.shape[1]] = blk
                off += blk.shape[1]
            wdT = _q8(w_down[e].T)                       # [F, H]
            for h in range(2):
                wdn[s, :, h * N_KF * 1024:(h + 1) * N_KF * 1024] = _sbufize(
                    wdT[:, h * 1024:(h + 1) * 1024], N_KF
                )
            toks = lists[e]
            n = len(toks)
            if n:
                xte = np.zeros((HIDDEN, C), dtype=np.float32)
                xte[:, :n] = xT[:, toks]
                xts[s] = _sbufize(xte.astype(BF16), N_KH)
                # per-token routing weights in expert order; the 1/WSCALE^2
                # undoes the u-side and down-side weight scales
                wcol = np.zeros(C, dtype=np.float32)
                for i in range(TOP_K):
                    sel = ids[:, i] == e
                    wcol[pos[sel, i]] = tw[sel, i]
                wv[:, s] = wcol / (WSCALE * WSCALE)
        # shared expert slice (tensor-parallel on intermediate dim)
        g_sl = ws_gate_up[c * SFS:(c + 1) * SFS]            # [352, H]
        u_sl = ws_gate_up[SHARED_FFN + c * SFS: SHARED_FFN + (c + 1) * SFS]
        wsgu = _sbufize(
            np.concatenate([g_sl, u_sl], axis=0).T.astype(BF16), N_KH
        )  # [128, 16*704]
        wsdT = ws_down[:, c * SFS:(c + 1) * SFS].T.astype(BF16)  # [352, H]
        wsd_pad = np.zeros((384, HIDDEN), dtype=BF16)
        wsd_pad[:SFS] = wsdT
        wsd = _sbufize(wsd_pad, 3)                          # [128, 3*2048]
        xsh = _sbufize(xT.astype(BF16), N_KH)               # [128, 16*256]
        in_maps.append({
            "xt": xts, "wgu": wgu, "wdn": wdn, "wv": wv,
            "xsh": xsh, "wsgu": wsgu, "wsd": wsd,
        })

    if C not in _PROGRAM_CACHE:
        _PROGRAM_CACHE[C] = _build_program(C)
    nc = _PROGRAM_CACHE[C]

    from concourse.bass_utils import run_bass_kernel_spmd
    res = run_bass_kernel_spmd(
        nc, in_maps, list(range(N_CORES)),
        trace=bool(os.environ.get("MOE_KERNEL_TRACE")),
    )
    LAST_RESULTS = res

    # ---- combine: gather-sum of weighted routed rows + shared partials ----
    y_all = np.stack([r["yrt"].astype(np.float32) for r in res.results])       # [8, EPC, C, H]
    y_flat = y_all.reshape(N_EXPERTS * C, HIDDEN)
    G = ids * C + pos                                       # [T, 6]
    routed = y_flat[G].sum(axis=1)
    shared = np.sum([r["ysh"].astype(np.float32) for r in res.results], axis=0)
    out = routed + shared
    return out.reshape(1, T, HIDDEN).astype(np.float32)



# revision 23
# speedup vs baseline: 1.0064x; 1.0064x over previous
"""DeepSeek-V2-Lite MoE layer on 8 Trainium2 NeuronCores.

Strategy (expert-parallel, per the sharding hint):
  - Host computes the gate (256x64 matmul + softmax + top-6) in fp32 numpy --
    this is the token dispatch decision, which necessarily lives on the host
    since the host builds the per-core input shards ("all-to-all" realized as
    host-side gather/scatter under the full-IO contract).
  - Each core owns 8 routed experts (weights sharded on the expert axis) and
    a 1/8 slice of the shared expert intermediate dim (tensor-parallel).
  - Tokens routed to each expert are gathered host-side into a fixed-capacity
    [C] batch (C = max expert load rounded up; uniform so the single SPMD
    program is identical across cores).
  - On device, all matmuls are token-stationary: the (small) token batch is
    the stationary PE operand, the expert weights stream through as the
    moving operand, so PE time ~ weight-columns/2.4GHz and the kernel is
    bound by the irreducible weight DMA (~138 MB/core in bf16).
  - Device applies the per-token routing weight; host combine is a pure
    gather-sum plus the 8-way shared-expert partial sum.
"""

import os
import numpy as np
import ml_dtypes

BF16 = ml_dtypes.bfloat16
F8E3 = ml_dtypes.float8_e3m4   # TRN FP8_EXP3: 4 mantissa bits, max 15.5

HIDDEN = 2048
FFN = 1408
N_EXPERTS = 64
TOP_K = 6
SHARED_FFN = 2816          # 2 shared experts * FFN
T = 256
N_CORES = 8
EPC = N_EXPERTS // N_CORES  # experts per core = 8
SFS = SHARED_FFN // N_CORES  # shared-FFN slice per core = 352

# routed expert weights are shipped in fp8-e3m4 scaled by WSCALE (so the
# +/-0.1-ish gaussian weights land in e3m4's normal range); the gate-side
# 1/WSCALE is undone inside the silu activation, the remaining WSCALE^2
# factor is folded into the per-token combine weights.
WSCALE = 64.0

# gate/up column interleave: stream order [g0|u0|g1|u1|g2|u2], pair widths
PAIR_W = [512, 512, 384]
PAIR_OFF = [0, 1024, 2048]          # start col of each pair block (2*w wide)
N_KH = HIDDEN // 128                # 16 K-chunks over hidden
N_KF = FFN // 128                   # 11 K-chunks over FFN

_PROGRAM_CACHE = {}
LAST_RESULTS = None


def _route(x, gate_w):
    """fp32 softmax top-k routing, matching jax.lax.top_k tie-breaking
    (stable sort -> lowest index wins ties)."""
    logits = x @ gate_w.T                      # [T, E] fp32
    m = logits.max(axis=-1, keepdims=True)
    e = np.exp(logits - m)
    scores = e / e.sum(axis=-1, keepdims=True)
    ids = np.argsort(-scores, axis=-1, kind="stable")[:, :TOP_K]
    w = np.take_along_axis(scores, ids, axis=-1)
    w = w / (w.sum(axis=-1, keepdims=True) + 1e-20)
    return ids, w.astype(np.float32)


def _build_program(C):
    import concourse.bass as bass
    import concourse.bacc as bacc
    import concourse.mybir as mybir
    import concourse.tile as tile
    from concourse.masks import make_identity
    from contextlib import ExitStack

    f32 = mybir.dt.float32
    bf16 = mybir.dt.bfloat16
    SILU = mybir.ActivationFunctionType.Silu
    COPY = mybir.ActivationFunctionType.Copy

    # Bacc (not plain Bass): its compile pipeline splits multi-wait
    # instructions into the 1-wait-per-instruction form TRN2 requires.
    nc = bacc.Bacc(None)

    f8 = mybir.dt.float8e3

    # DRAM layouts are host-prepped into final SBUF layouts so every weight
    # DMA is contiguous per partition row.
    W_GU = N_KH * 2816
    W_DN = 2 * N_KF * 1024
    d_xt = nc.dram_tensor("xt", [EPC, 128, N_KH * C], bf16, kind="ExternalInput")
    d_wgu = nc.dram_tensor("wgu", [EPC, 128, W_GU], f8, kind="ExternalInput")
    d_wdn = nc.dram_tensor("wdn", [EPC, 128, W_DN], f8, kind="ExternalInput")
    d_wv = nc.dram_tensor("wv", [C, EPC], f32, kind="ExternalInput")
    d_xsh = nc.dram_tensor("xsh", [128, N_KH * 256], bf16, kind="ExternalInput")
    d_wsgu = nc.dram_tensor("wsgu", [128, N_KH * 2 * SFS], bf16, kind="ExternalInput")
    d_wsd = nc.dram_tensor("wsd", [128, 3 * 2048], bf16, kind="ExternalInput")
    d_yrt = nc.dram_tensor("yrt", [EPC, C, HIDDEN], bf16, kind="ExternalOutput")
    d_ysh = nc.dram_tensor("ysh", [T, HIDDEN], bf16, kind="ExternalOutput")

    with tile.TileContext(nc) as tc, ExitStack() as ctx:
        p_const = ctx.enter_context(tc.tile_pool(name="const", bufs=1))
        p_wgu = ctx.enter_context(tc.tile_pool(name="wgu", bufs=3))
        p_wdn = ctx.enter_context(tc.tile_pool(name="wdn", bufs=4))
        p_wsh = ctx.enter_context(tc.tile_pool(name="wsh", bufs=1))
        p_xt = ctx.enter_context(tc.tile_pool(name="xt", bufs=2))
        p_act = ctx.enter_context(tc.tile_pool(name="act", bufs=2))
        p_gs = ctx.enter_context(tc.tile_pool(name="gs", bufs=2))
        p_actT = ctx.enter_context(tc.tile_pool(name="actT", bufs=2))
        p_out = ctx.enter_context(tc.tile_pool(name="out", bufs=2))
        # gate/up and down projections share one PSUM ring (same tag) so the
        # down matmuls never wait on a drain of their own dedicated buffer
        ps_gu = ctx.enter_context(tc.tile_pool(name="ps_gu", bufs=2, space="PSUM"))
        ps_t = ctx.enter_context(tc.tile_pool(name="ps_t", bufs=2, space="PSUM"))

        ident = p_const.tile([128, 128], bf16)
        make_identity(nc, ident[:])
        wv_t = p_const.tile([C, EPC], f32)
        nc.sync.dma_start(out=wv_t[:], in_=d_wv[:])

        state = {}

        def routed_gu(s):
            """gate+up matmuls and silu*u for expert slot s"""
            xt = p_xt.tile([128, N_KH * C], bf16, tag="xt")
            nc.sync.dma_start(out=xt[:], in_=d_xt[s])
            act = p_act.tile([C, FFN], bf16, tag="act")
            # ---- gate+up, pair-major over 3 (g,u) column pairs ----
            for pr in range(3):
                w = PAIR_W[pr]
                if s == 0 and pr == 0:
                    # first weights: one tile per pair of k-chunks so the PE
                    # starts as soon as the first 256KB lands (tile-granular
                    # dependency tracking would otherwise wait for all 2MB)
                    wgs = []
                    for q in range(8):
                        wq = p_wgu.tile([128, 2 * 2 * w], f8, tag=f"wgu0_{q}",
                                        bufs=1)
                        nc.sync.dma_start(
                            out=wq[:],
                            in_=d_wgu[s, :, q * 4 * w:(q + 1) * 4 * w],
                        )
                        wgs.append(wq)
                    wgof = lambda k: (wgs[k // 2], (k % 2) * 2 * w)
                else:
                    wg = p_wgu.tile([128, N_KH * 2 * w], f8, tag="wgu")
                    nc.sync.dma_start(
                        out=wg[:],
                        in_=d_wgu[s, :, N_KH * PAIR_OFF[pr]:
                                 N_KH * (PAIR_OFF[pr] + 2 * w)],
                    )
                    wgof = lambda k: (wg, k * 2 * w)
                pg = ps_gu.tile([C, 1024], mybir.dt.float32, tag="pg")
                for k in range(N_KH):
                    lhs = xt[:, k * C:(k + 1) * C]
                    wt, o = wgof(k)
                    nc.tensor.matmul(
                        pg[:, 0:w], lhs, wt[:, o:o + w],
                        start=(k == 0), stop=(k == N_KH - 1),
                    )
                    nc.tensor.matmul(
                        pg[:, 512:512 + w], lhs, wt[:, o + w:o + 2 * w],
                        start=(k == 0), stop=(k == N_KH - 1),
                    )
                gs = p_gs.tile([C, 512], mybir.dt.float32, tag="gs")
                # silu(G~ / WSCALE) undoes the gate-side weight scale exactly
                nc.scalar.activation(gs[:, :w], pg[:, :w], SILU, scale=1.0 / WSCALE)
                nc.vector.tensor_mul(
                    act[:, pr * 512: pr * 512 + w], gs[:, :w], pg[:, 512:512 + w]
                )
            # prefetch down weights now so they stream during the next gu block
            wds = []
            for h in range(2):
                wd = p_wdn.tile([128, N_KF * 1024], f8, tag="wdn")
                nc.sync.dma_start(
                    out=wd[:],
                    in_=d_wdn[s, :, h * N_KF * 1024:(h + 1) * N_KF * 1024],
                )
                wds.append(wd)
            state[s] = (act, wds)

        def routed_tail(s):
            """transpose + down projection + drain for expert slot s"""
            act, wds = state.pop(s)
            # ---- transpose act (tokens->free) for the down matmul ----
            # chunk transposes land in grouped PSUM tiles; 3 groups so the
            # first group's DVE copy drains while the PE transposes the rest
            # (the first down matmul then never waits on the copy latency)
            actT = p_actT.tile([128, N_KF * C], bf16, tag="actT")
            TG = (N_KF + 2) // 3
            j = 0
            while j < N_KF:
                g = min(TG, N_KF - j)
                pt = ps_t.tile([128, 512], bf16, tag="pt")
                for i in range(g):
                    nc.tensor.transpose(
                        pt[:, i * C:(i + 1) * C],
                        act[:, (j + i) * 128:(j + i + 1) * 128], ident[:C, :C]
                    )
                # drain on the (nearly idle) scalar engine: the DVE is blocked
                # behind the next expert's silu*u muls at exactly this moment
                nc.scalar.activation(actT[:, j * C:(j + g) * C], pt[:, :g * C],
                                     COPY)
                j += g
            # ---- down projection, N-half major ----
            out_sb = p_out.tile([C, HIDDEN], bf16, tag="out")
            for h in range(2):
                py = ps_gu.tile([C, 1024], mybir.dt.float32, tag="pg")
                for k in range(N_KF):
                    for n in range(2):
                        nc.tensor.matmul(
                            py[:, n * 512:(n + 1) * 512],
                            actT[:, k * C:(k + 1) * C],
                            wds[h][:, k * 1024 + n * 512: k * 1024 + (n + 1) * 512],
                            start=(k == 0), stop=(k == N_KF - 1),
                        )
                # routed combine weight folded in during PSUM drain
                nc.vector.tensor_scalar_mul(
                    out_sb[:, h * 1024:(h + 1) * 1024], py[:], wv_t[:, s:s + 1]
                )
            nc.sync.dma_start(out=d_yrt[s], in_=out_sb[:])

        def shared_expert():
            xsh = p_xt.tile([128, N_KH * 256], bf16, tag="xsh")
            nc.sync.dma_start(out=xsh[:], in_=d_xsh[:])
            wsg = p_wsh.tile([128, N_KH * 2 * SFS], bf16, tag="wsg")
            nc.sync.dma_start(out=wsg[:], in_=d_wsgu[:])
            wsd = p_wsh.tile([128, 3 * 2048], bf16, tag="wsd")
            nc.sync.dma_start(out=wsd[:], in_=d_wsd[:])
            for g in range(2):  # two groups of 128 tokens
                pg = ps_gu.tile([128, 1024], mybir.dt.float32, tag="pg")
                for k in range(N_KH):
                    lhs = xsh[:, k * 256 + g * 128: k * 256 + g * 128 + 128]
                    nc.tensor.matmul(
                        pg[:, 0:SFS], lhs, wsg[:, k * 2 * SFS: k * 2 * SFS + SFS],
                        start=(k == 0), stop=(k == N_KH - 1),
                    )
                    nc.tensor.matmul(
                        pg[:, 512:512 + SFS],
                        lhs, wsg[:, k * 2 * SFS + SFS: (k + 1) * 2 * SFS],
                        start=(k == 0), stop=(k == N_KH - 1),
                    )
                gs = p_gs.tile([128, 512], mybir.dt.float32, tag="gs")
                nc.scalar.activation(gs[:, :SFS], pg[:, :SFS], SILU)
                act_sh = p_act.tile([128, SFS], bf16, tag="act")
                nc.vector.tensor_mul(act_sh[:], gs[:, :SFS], pg[:, 512:512 + SFS])
                actT_sh = p_actT.tile([128, 3 * 128], bf16, tag="actT")
                # rows 96:128 of the last K-chunk pair with zero weight rows;
                # zero them so junk*0 can't produce NaN
                nc.vector.memset(actT_sh[:], 0.0)
                for j, wj in enumerate([128, 128, 96]):
                    pt = ps_t.tile([128, 128], bf16, tag="pt")
                    nc.tensor.transpose(
                        pt[:wj, :], act_sh[:, j * 128: j * 128 + wj], ident[:, :]
                    )
                    nc.scalar.activation(
                        actT_sh[:wj, j * 128:(j + 1) * 128], pt[:wj, :], COPY
                    )
                out_sh = p_out.tile([128, HIDDEN], bf16, tag="out")
                for h in range(2):
                    py = ps_gu.tile([128, 1024], mybir.dt.float32, tag="pg")
                    for k in range(3):
                        for n in range(2):
                            nc.tensor.matmul(
                                py[:, n * 512:(n + 1) * 512],
                                actT_sh[:, k * 128:(k + 1) * 128],
                                wsd[:, k * 2048 + h * 1024 + n * 512:
                                    k * 2048 + h * 1024 + (n + 1) * 512],
                                start=(k == 0), stop=(k == 2),
                            )
                    nc.vector.tensor_copy(out_sh[:, h * 1024:(h + 1) * 1024], py[:])
                nc.sync.dma_start(out=d_ysh[g * 128:(g + 1) * 128, :], in_=out_sh[:])

        # one-expert software-pipeline skew: the silu->mul->transpose chain of
        # expert s hides under expert s+1's gate/up matmuls; the shared expert
        # fills the same latency for the last slot and keeps the kernel tail
        # down to a small routed-output drain
        routed_gu(0)
        for s in range(1, EPC):
            routed_gu(s)
            routed_tail(s - 1)
        shared_expert()
        routed_tail(EPC - 1)

    if not nc.is_finalized():
        nc.finalize()
    return nc


def _sbufize(a, kdim):
    """[K*128, N] -> [128, K*N] SBUF layout (K-chunks along free dim)."""
    K = a.shape[0] // 128
    return np.ascontiguousarray(
        a.reshape(K, 128, -1).transpose(1, 0, 2).reshape(128, -1)
    )


def kernel(hidden_states, gate_w, w_gate_up, w_down, ws_gate_up, ws_down):
    global LAST_RESULTS
    x = np.asarray(hidden_states, dtype=np.float32).reshape(T, HIDDEN)
    gate_w = np.asarray(gate_w, dtype=np.float32)

    ids, tw = _route(x, gate_w)

    # per-expert token lists + positions
    lists = [[] for _ in range(N_EXPERTS)]
    pos = np.zeros((T, TOP_K), dtype=np.int64)
    for t in range(T):
        for i in range(TOP_K):
            e = ids[t, i]
            pos[t, i] = len(lists[e])
            lists[e].append(t)
    maxload = max(len(l) for l in lists)
    C = max(32, -(-maxload // 16) * 16)
    assert C <= 128, f"expert overload {maxload}: splitting not implemented"

    xb = x.astype(BF16)
    xT = np.ascontiguousarray(x.T)  # fp32 [H, T]

    # column permutation interleaving gate/up rows into [g0|u0|g1|u1|g2|u2]
    perm = np.concatenate([
        np.concatenate([np.arange(o, o + w), FFN + np.arange(o, o + w)])
        for o, w in zip([0, 512, 1024], PAIR_W)
    ])

    w_gate_up = np.asarray(w_gate_up)
    w_down = np.asarray(w_down)
    ws_gate_up = np.asarray(ws_gate_up)
    ws_down = np.asarray(ws_down)

    def _q8(a):
        return np.clip(a * WSCALE, -15.5, 15.5).astype(F8E3)

    in_maps = []
    for c in range(N_CORES):
        # routed expert weights (fp8-e3m4, scaled), token batches (bf16)
        wgu = np.empty((EPC, 128, N_KH * 2816), dtype=F8E3)
        wdn = np.empty((EPC, 128, 2 * N_KF * 1024), dtype=F8E3)
        xts = np.zeros((EPC, 128, N_KH * C), dtype=BF16)
        wv = np.zeros((C, EPC), dtype=np.float32)
        for s in range(EPC):
            e = c * EPC + s
            wt = _q8(w_gate_up[e][perm].T)              # [H, 2816] interleaved
            off = 0
            for o, w in zip(PAIR_OFF, PAIR_W):
                blk = _sbufize(wt[:, o:o + 2 * w], N_KH)  # [128, 16*2w]
                wgu[s, :, off:off + blk.shape[1]] = blk
                off += blk.shape[1]
            wdT = _q8(w_down[e].T)                       # [F, H]
            for h in range(2):
                wdn[s, :, h * N_KF * 1024:(h + 1) * N_KF * 1024] = _sbufize(
                    wdT[:, h * 1024:(h + 1) * 1024], N_KF
                )
            toks = lists[e]
            n = len(toks)
            if n:
                xte = np.zeros((HIDDEN, C), dtype=np.float32)
                xte[:, :n] = xT[:, toks]
                xts[s] = _sbufize(xte.astype(BF16), N_KH)
                # per-token routing weights in expert order; the 1/WSCALE^2
                # undoes the u-side and down-side weight scales
                wcol = np.zeros(C, dtype=np.float32)
                for i in range(TOP_K):
                    sel = ids[:, i] == e
                    wcol[pos[sel, i]] = tw[sel, i]
                wv[:, s] = wcol / (WSCALE * WSCALE)
        # shared expert slice (tensor-parallel on intermediate dim)
        g_sl = ws_gate_up[c * SFS:(c + 1) * SFS]            # [352, H]
        u_sl = ws_gate_up[SHARED_FFN + c * SFS: SHARED_FFN + (c + 1) * SFS]
        wsgu = _sbufize(
            np.concatenate([g_sl, u_sl], axis=0).T.astype(BF16), N_KH
        )  # [128, 16*704]
        wsdT = ws_down[:, c * SFS:(c + 1) * SFS].T.astype(BF16)  # [352, H]
        wsd_pad = np.zeros((384, HIDDEN), dtype=BF16)
        wsd_pad[:SFS] = wsdT
        wsd = _sbufize(wsd_pad, 3)                          # [128, 3*2048]
        xsh = _sbufize(xT.astype(BF16), N_KH)               # [128, 16*256]
        in_maps.append({
            "xt": xts, "wgu": wgu, "wdn": wdn, "wv": wv,
            "xsh": xsh, "wsgu": wsgu, "wsd": wsd,
        })

    if C not in _PROGRAM_CACHE:
        _PROGRAM_CACHE[C] = _build_program(C)
    nc = _PROGRAM_CACHE[C]

    from concourse.bass_utils import run_bass_kernel_spmd
    res = run_bass_kernel_spmd(
        nc, in_maps, list(range(N_CORES)),
        trace=bool(os.environ.get("MOE_KERNEL_TRACE")),
    )
    LAST_RESULTS = res

    # ---- combine: gather-sum of weighted routed rows + shared partials ----
    y_all = np.stack([r["yrt"].astype(np.float32) for r in res.results])       # [8, EPC, C, H]
    y_flat = y_all.reshape(N_EXPERTS * C, HIDDEN)
    G = ids * C + pos                                       # [T, 6]
    routed = y_flat[G].sum(axis=1)
    shared = np.sum([r["ysh"].astype(np.float32) for r in res.results], axis=0)
    out = routed + shared
    return out.reshape(1, T, HIDDEN).astype(np.float32)



# revision 30
# speedup vs baseline: 1.0119x; 1.0054x over previous
"""DeepSeek-V2-Lite MoE layer on 8 Trainium2 NeuronCores.

Strategy (expert-parallel, per the sharding hint):
  - Host computes the gate (256x64 matmul + softmax + top-6) in fp32 numpy --
    this is the token dispatch decision, which necessarily lives on the host
    since the host builds the per-core input shards ("all-to-all" realized as
    host-side gather/scatter under the full-IO contract).
  - Each core owns 8 routed experts (weights sharded on the expert axis) and
    a 1/8 slice of the shared expert intermediate dim (tensor-parallel).
  - Tokens routed to each expert are gathered host-side into a fixed-capacity
    [C] batch (C = max expert load rounded up; uniform so the single SPMD
    program is identical across cores).
  - On device, all matmuls are token-stationary: the (small) token batch is
    the stationary PE operand, the expert weights stream through as the
    moving operand, so PE time ~ weight-columns/2.4GHz and the kernel is
    bound by the irreducible weight DMA (~138 MB/core in bf16).
  - Device applies the per-token routing weight; host combine is a pure
    gather-sum plus the 8-way shared-expert partial sum.
"""

import os
import numpy as np
import ml_dtypes

BF16 = ml_dtypes.bfloat16
F8E3 = ml_dtypes.float8_e3m4   # TRN FP8_EXP3: 4 mantissa bits, max 15.5

HIDDEN = 2048
FFN = 1408
N_EXPERTS = 64
TOP_K = 6
SHARED_FFN = 2816          # 2 shared experts * FFN
T = 256
N_CORES = 8
EPC = N_EXPERTS // N_CORES  # experts per core = 8
SFS = SHARED_FFN // N_CORES  # shared-FFN slice per core = 352

# routed expert weights are shipped in fp8-e3m4 scaled by WSCALE (so the
# +/-0.1-ish gaussian weights land in e3m4's normal range); the gate-side
# 1/WSCALE is undone inside the silu activation, the remaining WSCALE^2
# factor is folded into the per-token combine weights.
WSCALE = 64.0

# gate/up column interleave: stream order [g0|u0|g1|u1|g2|u2], pair widths
PAIR_W = [512, 512, 384]
PAIR_OFF = [0, 1024, 2048]          # start col of each pair block (2*w wide)
N_KH = HIDDEN // 128                # 16 K-chunks over hidden
N_KF = FFN // 128                   # 11 K-chunks over FFN

_PROGRAM_CACHE = {}
LAST_RESULTS = None


def _route(x, gate_w):
    """fp32 softmax top-k routing, matching jax.lax.top_k tie-breaking
    (stable sort -> lowest index wins ties)."""
    logits = x @ gate_w.T                      # [T, E] fp32
    m = logits.max(axis=-1, keepdims=True)
    e = np.exp(logits - m)
    scores = e / e.sum(axis=-1, keepdims=True)
    ids = np.argsort(-scores, axis=-1, kind="stable")[:, :TOP_K]
    w = np.take_along_axis(scores, ids, axis=-1)
    w = w / (w.sum(axis=-1, keepdims=True) + 1e-20)
    return ids, w.astype(np.float32)


def _build_program(C):
    import concourse.bass as bass
    import concourse.bacc as bacc
    import concourse.mybir as mybir
    import concourse.tile as tile
    from concourse.masks import make_identity
    from contextlib import ExitStack

    f32 = mybir.dt.float32
    bf16 = mybir.dt.bfloat16
    SILU = mybir.ActivationFunctionType.Silu
    COPY = mybir.ActivationFunctionType.Copy

    # Bacc (not plain Bass): its compile pipeline splits multi-wait
    # instructions into the 1-wait-per-instruction form TRN2 requires.
    nc = bacc.Bacc(None)

    f8 = mybir.dt.float8e3

    # DRAM layouts are host-prepped into final SBUF layouts so every weight
    # DMA is contiguous per partition row.
    W_GU = N_KH * 2816
    W_DN = 2 * N_KF * 1024
    d_xt = nc.dram_tensor("xt", [EPC, 128, N_KH * C], bf16, kind="ExternalInput")
    d_wgu = nc.dram_tensor("wgu", [EPC, 128, W_GU], f8, kind="ExternalInput")
    d_wdn = nc.dram_tensor("wdn", [EPC, 128, W_DN], f8, kind="ExternalInput")
    d_wv = nc.dram_tensor("wv", [C, EPC], f32, kind="ExternalInput")
    d_xsh = nc.dram_tensor("xsh", [128, N_KH * 256], bf16, kind="ExternalInput")
    d_wsgu = nc.dram_tensor("wsgu", [128, N_KH * 2 * SFS], bf16, kind="ExternalInput")
    d_wsd = nc.dram_tensor("wsd", [128, 3 * 2048], bf16, kind="ExternalInput")
    d_yrt = nc.dram_tensor("yrt", [EPC, C, HIDDEN], bf16, kind="ExternalOutput")
    d_ysh = nc.dram_tensor("ysh", [T, HIDDEN], bf16, kind="ExternalOutput")

    with tile.TileContext(nc) as tc, ExitStack() as ctx:
        p_const = ctx.enter_context(tc.tile_pool(name="const", bufs=1))
        p_wgu = ctx.enter_context(tc.tile_pool(name="wgu", bufs=3))
        p_wdn = ctx.enter_context(tc.tile_pool(name="wdn", bufs=4))
        p_wsh = ctx.enter_context(tc.tile_pool(name="wsh", bufs=1))
        p_xt = ctx.enter_context(tc.tile_pool(name="xt", bufs=2))
        p_act = ctx.enter_context(tc.tile_pool(name="act", bufs=2))
        p_gs = ctx.enter_context(tc.tile_pool(name="gs", bufs=2))
        p_actT = ctx.enter_context(tc.tile_pool(name="actT", bufs=2))
        p_out = ctx.enter_context(tc.tile_pool(name="out", bufs=2))
        # gate/up and down projections share one PSUM ring (same tag) so the
        # down matmuls never wait on a drain of their own dedicated buffer
        ps_gu = ctx.enter_context(tc.tile_pool(name="ps_gu", bufs=2, space="PSUM"))
        ps_t = ctx.enter_context(tc.tile_pool(name="ps_t", bufs=2, space="PSUM"))

        ident = p_const.tile([128, 128], bf16)
        make_identity(nc, ident[:])
        wv_t = p_const.tile([C, EPC], f32)
        nc.sync.dma_start(out=wv_t[:], in_=d_wv[:])

        state = {}

        def routed_gu(s):
            """gate+up matmuls and silu*u for expert slot s"""
            xt = p_xt.tile([128, N_KH * C], bf16, tag="xt")
            nc.sync.dma_start(out=xt[:], in_=d_xt[s])
            act = p_act.tile([C, FFN], bf16, tag="act")
            # ---- gate+up, pair-major over 3 (g,u) column pairs ----
            for pr in range(3):
                w = PAIR_W[pr]
                if s == 0 and pr == 0:
                    # first weights: one tile per pair of k-chunks so the PE
                    # starts as soon as the first 256KB lands (tile-granular
                    # dependency tracking would otherwise wait for all 2MB)
                    wgs = []
                    for q in range(8):
                        wq = p_wgu.tile([128, 2 * 2 * w], f8, tag=f"wgu0_{q}",
                                        bufs=1)
                        nc.sync.dma_start(
                            out=wq[:],
                            in_=d_wgu[s, :, q * 4 * w:(q + 1) * 4 * w],
                        )
                        wgs.append(wq)
                    wgof = lambda k: (wgs[k // 2], (k % 2) * 2 * w)
                else:
                    wg = p_wgu.tile([128, N_KH * 2 * w], f8, tag="wgu")
                    nc.sync.dma_start(
                        out=wg[:],
                        in_=d_wgu[s, :, N_KH * PAIR_OFF[pr]:
                                 N_KH * (PAIR_OFF[pr] + 2 * w)],
                    )
                    wgof = lambda k: (wg, k * 2 * w)
                pg = ps_gu.tile([C, 1024], mybir.dt.float32, tag="pg")
                for k in range(N_KH):
                    lhs = xt[:, k * C:(k + 1) * C]
                    wt, o = wgof(k)
                    # matmul output is capped at one PSUM bank (512 fp32)
                    nc.tensor.matmul(
                        pg[:, 0:w], lhs, wt[:, o:o + w],
                        start=(k == 0), stop=(k == N_KH - 1),
                    )
                    nc.tensor.matmul(
                        pg[:, 512:512 + w], lhs, wt[:, o + w:o + 2 * w],
                        start=(k == 0), stop=(k == N_KH - 1),
                    )
                gs = p_gs.tile([C, 512], mybir.dt.float32, tag="gs")
                # silu(G~ / WSCALE) undoes the gate-side weight scale exactly
                nc.scalar.activation(gs[:, :w], pg[:, :w], SILU, scale=1.0 / WSCALE)
                nc.vector.tensor_mul(
                    act[:, pr * 512: pr * 512 + w], gs[:, :w], pg[:, 512:512 + w]
                )
            # prefetch down weights now so they stream during the next gu block
            wds = []
            for h in range(2):
                wd = p_wdn.tile([128, N_KF * 1024], f8, tag="wdn")
                nc.sync.dma_start(
                    out=wd[:],
                    in_=d_wdn[s, :, h * N_KF * 1024:(h + 1) * N_KF * 1024],
                )
                wds.append(wd)
            state[s] = (act, wds)

        def routed_tail(s):
            """transpose + down projection + drain for expert slot s"""
            act, wds = state.pop(s)
            # ---- transpose act (tokens->free) for the down matmul ----
            # chunk transposes land in grouped PSUM tiles; 3 groups so the
            # first group's DVE copy drains while the PE transposes the rest
            # (the first down matmul then never waits on the copy latency)
            actT = p_actT.tile([128, N_KF * C], bf16, tag="actT")
            TG = (N_KF + 2) // 3
            j = 0
            while j < N_KF:
                g = min(TG, N_KF - j)
                pt = ps_t.tile([128, 512], bf16, tag="pt")
                for i in range(g):
                    nc.tensor.transpose(
                        pt[:, i * C:(i + 1) * C],
                        act[:, (j + i) * 128:(j + i + 1) * 128], ident[:C, :C]
                    )
                # drain on the (nearly idle) scalar engine: the DVE is blocked
                # behind the next expert's silu*u muls at exactly this moment
                nc.scalar.activation(actT[:, j * C:(j + g) * C], pt[:, :g * C],
                                     COPY)
                j += g
            # ---- down projection, N-half major ----
            out_sb = p_out.tile([C, HIDDEN], bf16, tag="out")
            for h in range(2):
                py = ps_gu.tile([C, 1024], mybir.dt.float32, tag="pg")
                for k in range(N_KF):
                    for n in range(2):
                        nc.tensor.matmul(
                            py[:, n * 512:(n + 1) * 512],
                            actT[:, k * C:(k + 1) * C],
                            wds[h][:, k * 1024 + n * 512: k * 1024 + (n + 1) * 512],
                            start=(k == 0), stop=(k == N_KF - 1),
                        )
                # routed combine weight folded in during PSUM drain
                nc.vector.tensor_scalar_mul(
                    out_sb[:, h * 1024:(h + 1) * 1024], py[:], wv_t[:, s:s + 1]
                )
            nc.sync.dma_start(out=d_yrt[s], in_=out_sb[:])

        def shared_expert():
            xsh = p_xt.tile([128, N_KH * 256], bf16, tag="xsh")
            nc.sync.dma_start(out=xsh[:], in_=d_xsh[:])
            wsg = p_wsh.tile([128, N_KH * 2 * SFS], bf16, tag="wsg")
            nc.sync.dma_start(out=wsg[:], in_=d_wsgu[:])
            wsd = p_wsh.tile([128, 3 * 2048], bf16, tag="wsd")
            nc.sync.dma_start(out=wsd[:], in_=d_wsd[:])
            # both token groups' gate/up first so group 1's matmuls hide the
            # silu->mul->transpose chain of group 0
            acts = []
            for g in range(2):  # two groups of 128 tokens
                pg = ps_gu.tile([128, 1024], mybir.dt.float32, tag="pg")
                for k in range(N_KH):
                    lhs = xsh[:, k * 256 + g * 128: k * 256 + g * 128 + 128]
                    nc.tensor.matmul(
                        pg[:, 0:SFS], lhs, wsg[:, k * 2 * SFS: k * 2 * SFS + SFS],
                        start=(k == 0), stop=(k == N_KH - 1),
                    )
                    nc.tensor.matmul(
                        pg[:, 512:512 + SFS],
                        lhs, wsg[:, k * 2 * SFS + SFS: (k + 1) * 2 * SFS],
                        start=(k == 0), stop=(k == N_KH - 1),
                    )
                gs = p_gs.tile([128, 512], mybir.dt.float32, tag="gs")
                nc.scalar.activation(gs[:, :SFS], pg[:, :SFS], SILU)
                act_sh = p_act.tile([128, SFS], bf16, tag="acts")
                nc.vector.tensor_mul(act_sh[:], gs[:, :SFS], pg[:, 512:512 + SFS])
                acts.append(act_sh)
            for g in range(2):
                act_sh = acts[g]
                actT_sh = p_actT.tile([128, 3 * 128], bf16, tag="actTs")
                # rows 96:128 of the last K-chunk pair with zero weight rows;
                # zero them so junk*0 can't produce NaN
                nc.vector.memset(actT_sh[:], 0.0)
                for j, wj in enumerate([128, 128, 96]):
                    pt = ps_t.tile([128, 128], bf16, tag="pt")
                    nc.tensor.transpose(
                        pt[:wj, :], act_sh[:, j * 128: j * 128 + wj], ident[:, :]
                    )
                    nc.scalar.activation(
                        actT_sh[:wj, j * 128:(j + 1) * 128], pt[:wj, :], COPY
                    )
                out_sh = p_out.tile([128, HIDDEN], bf16, tag="outs")
                for h in range(2):
                    py = ps_gu.tile([128, 1024], mybir.dt.float32, tag="pg")
                    for k in range(3):
                        for n in range(2):
                            nc.tensor.matmul(
                                py[:, n * 512:(n + 1) * 512],
                                actT_sh[:, k * 128:(k + 1) * 128],
                                wsd[:, k * 2048 + h * 1024 + n * 512:
                                    k * 2048 + h * 1024 + (n + 1) * 512],
                                start=(k == 0), stop=(k == 2),
                            )
                    nc.vector.tensor_copy(out_sh[:, h * 1024:(h + 1) * 1024], py[:])
                nc.sync.dma_start(out=d_ysh[g * 128:(g + 1) * 128, :], in_=out_sh[:])

        # one-expert software-pipeline skew: the silu->mul->transpose chain of
        # expert s hides under expert s+1's gate/up matmuls; the shared expert
        # fills the same latency for the last slot and keeps the kernel tail
        # down to a small routed-output drain
        routed_gu(0)
        for s in range(1, EPC):
            routed_gu(s)
            routed_tail(s - 1)
        shared_expert()
        routed_tail(EPC - 1)

    if not nc.is_finalized():
        nc.finalize()
    return nc


def _sbufize(a, kdim):
    """[K*128, N] -> [128, K*N] SBUF layout (K-chunks along free dim)."""
    K = a.shape[0] // 128
    return np.ascontiguousarray(
        a.reshape(K, 128, -1).transpose(1, 0, 2).reshape(128, -1)
    )


def kernel(hidden_states, gate_w, w_gate_up, w_down, ws_gate_up, ws_down):
    global LAST_RESULTS
    x = np.asarray(hidden_states, dtype=np.float32).reshape(T, HIDDEN)
    gate_w = np.asarray(gate_w, dtype=np.float32)

    ids, tw = _route(x, gate_w)

    # per-expert token lists + positions
    lists = [[] for _ in range(N_EXPERTS)]
    pos = np.zeros((T, TOP_K), dtype=np.int64)
    for t in range(T):
        for i in range(TOP_K):
            e = ids[t, i]
            pos[t, i] = len(lists[e])
            lists[e].append(t)
    maxload = max(len(l) for l in lists)
    C = max(32, -(-maxload // 16) * 16)
    assert C <= 128, f"expert overload {maxload}: splitting not implemented"

    xb = x.astype(BF16)
    xT = np.ascontiguousarray(x.T)  # fp32 [H, T]

    # column permutation interleaving gate/up rows into [g0|u0|g1|u1|g2|u2]
    perm = np.concatenate([
        np.concatenate([np.arange(o, o + w), FFN + np.arange(o, o + w)])
        for o, w in zip([0, 512, 1024], PAIR_W)
    ])

    w_gate_up = np.asarray(w_gate_up)
    w_down = np.asarray(w_down)
    ws_gate_up = np.asarray(ws_gate_up)
    ws_down = np.asarray(ws_down)

    def _q8(a):
        return np.clip(a * WSCALE, -15.5, 15.5).astype(F8E3)

    in_maps = []
    for c in range(N_CORES):
        # routed expert weights (fp8-e3m4, scaled), token batches (bf16)
        wgu = np.empty((EPC, 128, N_KH * 2816), dtype=F8E3)
        wdn = np.empty((EPC, 128, 2 * N_KF * 1024), dtype=F8E3)
        xts = np.zeros((EPC, 128, N_KH * C), dtype=BF16)
        wv = np.zeros((C, EPC), dtype=np.float32)
        for s in range(EPC):
            e = c * EPC + s
            wt = _q8(w_gate_up[e][perm].T)              # [H, 2816] interleaved
            off = 0
            for o, w in zip(PAIR_OFF, PAIR_W):
                blk = _sbufize(wt[:, o:o + 2 * w], N_KH)  # [128, 16*2w]
                wgu[s, :, off:off + blk.shape[1]] = blk
                off += blk.shape[1]
            wdT = _q8(w_down[e].T)                       # [F, H]
            for h in range(2):
                wdn[s, :, h * N_KF * 1024:(h + 1) * N_KF * 1024] = _sbufize(
                    wdT[:, h * 1024:(h + 1) * 1024], N_KF
                )
            toks = lists[e]
            n = len(toks)
            if n:
                xte = np.zeros((HIDDEN, C), dtype=np.float32)
                xte[:, :n] = xT[:, toks]
                xts[s] = _sbufize(xte.astype(BF16), N_KH)
                # per-token routing weights in expert order; the 1/WSCALE^2
                # undoes the u-side and down-side weight scales
                wcol = np.zeros(C, dtype=np.float32)
                for i in range(TOP_K):
                    sel = ids[:, i] == e
                    wcol[pos[sel, i]] = tw[sel, i]
                wv[:, s] = wcol / (WSCALE * WSCALE)
        # shared expert slice (tensor-parallel on intermediate dim)
        g_sl = ws_gate_up[c * SFS:(c + 1) * SFS]            # [352, H]
        u_sl = ws_gate_up[SHARED_FFN + c * SFS: SHARED_FFN + (c + 1) * SFS]
        wsgu = _sbufize(
            np.concatenate([g_sl, u_sl], axis=0).T.astype(BF16), N_KH
        )  # [128, 16*704]
        wsdT = ws_down[:, c * SFS:(c + 1) * SFS].T.astype(BF16)  # [352, H]
        wsd_pad = np.zeros((384, HIDDEN), dtype=BF16)
        wsd_pad[:SFS] = wsdT
        wsd = _sbufize(wsd_pad, 3)                          # [128, 3*2048]
        xsh = _sbufize(xT.astype(BF16), N_KH)               # [128, 16*256]
        in_maps.append({
            "xt": xts, "wgu": wgu, "wdn": wdn, "wv": wv,
            "xsh": xsh, "wsgu": wsgu, "wsd": wsd,
        })

    if C not in _PROGRAM_CACHE:
        _PROGRAM_CACHE[C] = _build_program(C)
    nc = _PROGRAM_CACHE[C]

    from concourse.bass_utils import run_bass_kernel_spmd
    res = run_bass_kernel_spmd(
        nc, in_maps, list(range(N_CORES)),
        trace=bool(os.environ.get("MOE_KERNEL_TRACE")),
    )
    LAST_RESULTS = res

    # ---- combine: gather-sum of weighted routed rows + shared partials ----
    y_all = np.stack([r["yrt"].astype(np.float32) for r in res.results])       # [8, EPC, C, H]
    y_flat = y_all.reshape(N_EXPERTS * C, HIDDEN)
    G = ids * C + pos                                       # [T, 6]
    routed = y_flat[G].sum(axis=1)
    shared = np.sum([r["ysh"].astype(np.float32) for r in res.results], axis=0)
    out = routed + shared
    return out.reshape(1, T, HIDDEN).astype(np.float32)



# revision 34
# speedup vs baseline: 1.0615x; 1.0490x over previous
"""DeepSeek-V2-Lite MoE layer on 8 Trainium2 NeuronCores.

Strategy (expert-parallel, per the sharding hint):
  - Host computes the gate (256x64 matmul + softmax + top-6) in fp32 numpy --
    this is the token dispatch decision, which necessarily lives on the host
    since the host builds the per-core input shards ("all-to-all" realized as
    host-side gather/scatter under the full-IO contract).
  - Each core owns 8 routed experts (weights sharded on the expert axis) and
    a 1/8 slice of the shared expert intermediate dim (tensor-parallel).
  - Tokens routed to each expert are gathered host-side into a fixed-capacity
    [C] batch (C = max expert load rounded up; uniform so the single SPMD
    program is identical across cores).
  - On device, all matmuls are token-stationary: the (small) token batch is
    the stationary PE operand, the expert weights stream through as the
    moving operand, so PE time ~ weight-columns/2.4GHz and the kernel is
    bound by the irreducible weight DMA (~138 MB/core in bf16).
  - Device applies the per-token routing weight; host combine is a pure
    gather-sum plus the 8-way shared-expert partial sum.
"""

import os
import numpy as np
import ml_dtypes

BF16 = ml_dtypes.bfloat16
F8E3 = ml_dtypes.float8_e3m4   # TRN FP8_EXP3: 4 mantissa bits, max 15.5

HIDDEN = 2048
FFN = 1408
N_EXPERTS = 64
TOP_K = 6
SHARED_FFN = 2816          # 2 shared experts * FFN
T = 256
N_CORES = 8
EPC = N_EXPERTS // N_CORES  # experts per core = 8
SFS = SHARED_FFN // N_CORES  # shared-FFN slice per core = 352

# routed expert weights are shipped in fp8-e3m4 scaled by WSCALE (so the
# +/-0.1-ish gaussian weights land in e3m4's normal range); the gate-side
# 1/WSCALE is undone inside the silu activation, the remaining WSCALE^2
# factor is folded into the per-token combine weights.
WSCALE = 64.0

# gate/up column interleave: stream order [g0|u0|g1|u1|g2|u2], pair widths
PAIR_W = [512, 512, 384]
PAIR_OFF = [0, 1024, 2048]          # start col of each pair block (2*w wide)
N_KH = HIDDEN // 128                # 16 K-chunks over hidden
N_KF = FFN // 128                   # 11 K-chunks over FFN

_PROGRAM_CACHE = {}
LAST_RESULTS = None


def _route(x, gate_w):
    """fp32 softmax top-k routing, matching jax.lax.top_k tie-breaking
    (stable sort -> lowest index wins ties)."""
    logits = x @ gate_w.T                      # [T, E] fp32
    m = logits.max(axis=-1, keepdims=True)
    e = np.exp(logits - m)
    scores = e / e.sum(axis=-1, keepdims=True)
    ids = np.argsort(-scores, axis=-1, kind="stable")[:, :TOP_K]
    w = np.take_along_axis(scores, ids, axis=-1)
    w = w / (w.sum(axis=-1, keepdims=True) + 1e-20)
    return ids, w.astype(np.float32)


def _build_program(C):
    import concourse.bass as bass
    import concourse.bacc as bacc
    import concourse.mybir as mybir
    import concourse.tile as tile
    from concourse.masks import make_identity
    from contextlib import ExitStack

    f32 = mybir.dt.float32
    bf16 = mybir.dt.bfloat16
    SILU = mybir.ActivationFunctionType.Silu
    COPY = mybir.ActivationFunctionType.Copy

    # Bacc (not plain Bass): its compile pipeline splits multi-wait
    # instructions into the 1-wait-per-instruction form TRN2 requires.
    nc = bacc.Bacc(None)

    f8 = mybir.dt.float8e3

    # DRAM layouts are host-prepped into final SBUF layouts so every weight
    # DMA is contiguous per partition row.
    W_GU = N_KH * 2816
    W_DN = 2 * N_KF * 1024
    d_xt = nc.dram_tensor("xt", [EPC, 128, N_KH * C], bf16, kind="ExternalInput")
    d_wgu = nc.dram_tensor("wgu", [EPC, 128, W_GU], f8, kind="ExternalInput")
    d_wdn = nc.dram_tensor("wdn", [EPC, 128, W_DN], f8, kind="ExternalInput")
    d_wv = nc.dram_tensor("wv", [C, EPC], f32, kind="ExternalInput")
    d_xsh = nc.dram_tensor("xsh", [128, N_KH * 256], bf16, kind="ExternalInput")
    d_wsgu = nc.dram_tensor("wsgu", [128, N_KH * 2 * SFS], bf16, kind="ExternalInput")
    d_wsd = nc.dram_tensor("wsd", [128, 3 * 2048], bf16, kind="ExternalInput")
    d_yrt = nc.dram_tensor("yrt", [EPC, C, HIDDEN], bf16, kind="ExternalOutput")
    d_ysh = nc.dram_tensor("ysh", [T, HIDDEN], bf16, kind="ExternalOutput")

    with tile.TileContext(nc) as tc, ExitStack() as ctx:
        p_const = ctx.enter_context(tc.tile_pool(name="const", bufs=1))
        p_wgu = ctx.enter_context(tc.tile_pool(name="wgu", bufs=3))
        p_wdn = ctx.enter_context(tc.tile_pool(name="wdn", bufs=2))
        p_wsh = ctx.enter_context(tc.tile_pool(name="wsh", bufs=1))
        p_xt = ctx.enter_context(tc.tile_pool(name="xt", bufs=2))
        p_act = ctx.enter_context(tc.tile_pool(name="act", bufs=2))
        p_gs = ctx.enter_context(tc.tile_pool(name="gs", bufs=2))
        p_actT = ctx.enter_context(tc.tile_pool(name="actT", bufs=2))
        p_out = ctx.enter_context(tc.tile_pool(name="out", bufs=2))
        # gate/up and down projections share one PSUM ring (same tag) so the
        # down matmuls never wait on a drain of their own dedicated buffer
        ps_gu = ctx.enter_context(tc.tile_pool(name="ps_gu", bufs=2, space="PSUM"))
        ps_t = ctx.enter_context(tc.tile_pool(name="ps_t", bufs=3, space="PSUM"))

        ident = p_const.tile([128, 128], bf16)
        make_identity(nc, ident[:])
        wv_t = p_const.tile([C, EPC], f32)
        nc.sync.dma_start(out=wv_t[:], in_=d_wv[:])

        state = {}

        def issue_wd(s):
            """down-weight DMA for slot s; issued one gu-block before use so
            the queue stays deadline-ordered (gu pairs are more urgent)"""
            wds = []
            for h in range(2):
                wd = p_wdn.tile([128, N_KF * 1024], f8, tag="wdn")
                nc.sync.dma_start(
                    out=wd[:],
                    in_=d_wdn[s, :, h * N_KF * 1024:(h + 1) * N_KF * 1024],
                )
                wds.append(wd)
            state[s] = state[s] + (wds,)

        def routed_gu(s, mid=None):
            """gate+up matmuls and silu*u for expert slot s; mid() is emitted
            between pair 1 and pair 2 (the previous expert's transposes run
            there, so their scalar-engine copies beat silu-p2 in the queue)"""
            xt = p_xt.tile([128, N_KH * C], bf16, tag="xt")
            nc.sync.dma_start(out=xt[:], in_=d_xt[s])
            act = p_act.tile([C, FFN], bf16, tag="act")
            # ---- gate+up, pair-major over 3 (g,u) column pairs ----
            for pr in range(3):
                w = PAIR_W[pr]
                if s == 0 and pr == 0:
                    # first weights: one tile per pair of k-chunks so the PE
                    # starts as soon as the first 256KB lands (tile-granular
                    # dependency tracking would otherwise wait for all 2MB)
                    wgs = []
                    for q in range(8):
                        wq = p_wgu.tile([128, 2 * 2 * w], f8, tag=f"wgu0_{q}",
                                        bufs=1)
                        nc.sync.dma_start(
                            out=wq[:],
                            in_=d_wgu[s, :, q * 4 * w:(q + 1) * 4 * w],
                        )
                        wgs.append(wq)
                    wgof = lambda k: (wgs[k // 2], (k % 2) * 2 * w)
                else:
                    wg = p_wgu.tile([128, N_KH * 2 * w], f8, tag="wgu")
                    nc.sync.dma_start(
                        out=wg[:],
                        in_=d_wgu[s, :, N_KH * PAIR_OFF[pr]:
                                 N_KH * (PAIR_OFF[pr] + 2 * w)],
                    )
                    wgof = lambda k: (wg, k * 2 * w)
                if pr == 2 and mid is not None:
                    mid()
                pg = ps_gu.tile([C, 1024], mybir.dt.float32, tag="pg")
                for k in range(N_KH):
                    lhs = xt[:, k * C:(k + 1) * C]
                    wt, o = wgof(k)
                    # matmul output is capped at one PSUM bank (512 fp32)
                    nc.tensor.matmul(
                        pg[:, 0:w], lhs, wt[:, o:o + w],
                        start=(k == 0), stop=(k == N_KH - 1),
                    )
                    nc.tensor.matmul(
                        pg[:, 512:512 + w], lhs, wt[:, o + w:o + 2 * w],
                        start=(k == 0), stop=(k == N_KH - 1),
                    )
                gs = p_gs.tile([C, 512], mybir.dt.float32, tag="gs")
                # silu(G~ / WSCALE) undoes the gate-side weight scale exactly
                nc.scalar.activation(gs[:, :w], pg[:, :w], SILU, scale=1.0 / WSCALE)
                nc.vector.tensor_mul(
                    act[:, pr * 512: pr * 512 + w], gs[:, :w], pg[:, 512:512 + w]
                )
            state[s] = (act,)

        def routed_mid(s):
            """transpose act (tokens->free) of slot s for its down matmul"""
            act = state[s][0]
            actT = p_actT.tile([128, N_KF * C], bf16, tag="actT")
            TG = (N_KF + 2) // 3
            j = 0
            while j < N_KF:
                g = min(TG, N_KF - j)
                pt = ps_t.tile([128, 512], bf16, tag="pt")
                for i in range(g):
                    nc.tensor.transpose(
                        pt[:, i * C:(i + 1) * C],
                        act[:, (j + i) * 128:(j + i + 1) * 128], ident[:C, :C]
                    )
                # drain on the (nearly idle) scalar engine: the DVE is blocked
                # behind the next expert's silu*u muls at exactly this moment
                nc.scalar.activation(actT[:, j * C:(j + g) * C], pt[:, :g * C],
                                     COPY)
                j += g
            state[s] = state[s] + (actT,)

        def routed_down(s):
            """down projection + drain for expert slot s"""
            act, actT, wds = state.pop(s)
            out_sb = p_out.tile([C, HIDDEN], bf16, tag="out")
            for h in range(2):
                py = ps_gu.tile([C, 1024], mybir.dt.float32, tag="pg")
                for k in range(N_KF):
                    for n in range(2):
                        nc.tensor.matmul(
                            py[:, n * 512:(n + 1) * 512],
                            actT[:, k * C:(k + 1) * C],
                            wds[h][:, k * 1024 + n * 512: k * 1024 + (n + 1) * 512],
                            start=(k == 0), stop=(k == N_KF - 1),
                        )
                # combine weight folded into the drain; split across the DVE
                # and scalar engines so the two halves drain in parallel
                nc.vector.tensor_scalar_mul(
                    out_sb[:, h * 1024:h * 1024 + 512], py[:, 0:512],
                    wv_t[:, s:s + 1]
                )
                nc.scalar.activation(
                    out_sb[:, h * 1024 + 512:(h + 1) * 1024], py[:, 512:1024],
                    COPY, scale=wv_t[:, s:s + 1]
                )
            nc.sync.dma_start(out=d_yrt[s], in_=out_sb[:])

        def shared_expert():
            xsh = p_xt.tile([128, N_KH * 256], bf16, tag="xsh")
            nc.sync.dma_start(out=xsh[:], in_=d_xsh[:])
            wsg = p_wsh.tile([128, N_KH * 2 * SFS], bf16, tag="wsg")
            nc.sync.dma_start(out=wsg[:], in_=d_wsgu[:])
            wsd = p_wsh.tile([128, 3 * 2048], bf16, tag="wsd")
            nc.sync.dma_start(out=wsd[:], in_=d_wsd[:])
            # both token groups' gate/up first so group 1's matmuls hide the
            # silu->mul->transpose chain of group 0
            acts = []
            for g in range(2):  # two groups of 128 tokens
                pg = ps_gu.tile([128, 1024], mybir.dt.float32, tag="pg")
                for k in range(N_KH):
                    lhs = xsh[:, k * 256 + g * 128: k * 256 + g * 128 + 128]
                    nc.tensor.matmul(
                        pg[:, 0:SFS], lhs, wsg[:, k * 2 * SFS: k * 2 * SFS + SFS],
                        start=(k == 0), stop=(k == N_KH - 1),
                    )
                    nc.tensor.matmul(
                        pg[:, 512:512 + SFS],
                        lhs, wsg[:, k * 2 * SFS + SFS: (k + 1) * 2 * SFS],
                        start=(k == 0), stop=(k == N_KH - 1),
                    )
                gs = p_gs.tile([128, 512], mybir.dt.float32, tag="gs")
                nc.scalar.activation(gs[:, :SFS], pg[:, :SFS], SILU)
                act_sh = p_act.tile([128, SFS], bf16, tag="acts")
                nc.vector.tensor_mul(act_sh[:], gs[:, :SFS], pg[:, 512:512 + SFS])
                acts.append(act_sh)
            for g in range(2):
                act_sh = acts[g]
                actT_sh = p_actT.tile([128, 3 * 128], bf16, tag="actTs")
                # rows 96:128 of the last K-chunk pair with zero weight rows;
                # zero them so junk*0 can't produce NaN
                nc.vector.memset(actT_sh[:], 0.0)
                for j, wj in enumerate([128, 128, 96]):
                    pt = ps_t.tile([128, 128], bf16, tag="pt")
                    nc.tensor.transpose(
                        pt[:wj, :], act_sh[:, j * 128: j * 128 + wj], ident[:, :]
                    )
                    nc.scalar.activation(
                        actT_sh[:wj, j * 128:(j + 1) * 128], pt[:wj, :], COPY
                    )
                out_sh = p_out.tile([128, HIDDEN], bf16, tag="outs")
                for h in range(2):
                    py = ps_gu.tile([128, 1024], mybir.dt.float32, tag="pg")
                    for k in range(3):
                        for n in range(2):
                            nc.tensor.matmul(
                                py[:, n * 512:(n + 1) * 512],
                                actT_sh[:, k * 128:(k + 1) * 128],
                                wsd[:, k * 2048 + h * 1024 + n * 512:
                                    k * 2048 + h * 1024 + (n + 1) * 512],
                                start=(k == 0), stop=(k == 2),
                            )
                    nc.vector.tensor_copy(out_sh[:, h * 1024:(h + 1) * 1024], py[:])
                nc.sync.dma_start(out=d_ysh[g * 128:(g + 1) * 128, :], in_=out_sh[:])

        # one-expert software-pipeline skew: expert s-1's transposes slot in
        # between expert s's gu pairs 1 and 2 (so their scalar copies beat
        # silu-p2 in the scalar queue) and its down matmuls follow pair 2
        # immediately; DMA issue stays deadline-ordered (gu pairs first, then
        # the previous expert's down weights). The shared expert fills the
        # drain chain of the last slot and keeps the kernel tail small.
        routed_gu(0)
        for s in range(1, EPC):
            routed_gu(s, mid=lambda prev=s - 1: routed_mid(prev))
            issue_wd(s - 1)
            routed_down(s - 1)
        shared_expert()
        routed_mid(EPC - 1)
        issue_wd(EPC - 1)
        routed_down(EPC - 1)

    if not nc.is_finalized():
        nc.finalize()
    return nc


def _sbufize(a, kdim):
    """[K*128, N] -> [128, K*N] SBUF layout (K-chunks along free dim)."""
    K = a.shape[0] // 128
    return np.ascontiguousarray(
        a.reshape(K, 128, -1).transpose(1, 0, 2).reshape(128, -1)
    )


def kernel(hidden_states, gate_w, w_gate_up, w_down, ws_gate_up, ws_down):
    global LAST_RESULTS
    x = np.asarray(hidden_states, dtype=np.float32).reshape(T, HIDDEN)
    gate_w = np.asarray(gate_w, dtype=np.float32)

    ids, tw = _route(x, gate_w)

    # per-expert token lists + positions
    lists = [[] for _ in range(N_EXPERTS)]
    pos = np.zeros((T, TOP_K), dtype=np.int64)
    for t in range(T):
        for i in range(TOP_K):
            e = ids[t, i]
            pos[t, i] = len(lists[e])
            lists[e].append(t)
    maxload = max(len(l) for l in lists)
    C = max(32, -(-maxload // 16) * 16)
    assert C <= 128, f"expert overload {maxload}: splitting not implemented"

    xb = x.astype(BF16)
    xT = np.ascontiguousarray(x.T)  # fp32 [H, T]

    # column permutation interleaving gate/up rows into [g0|u0|g1|u1|g2|u2]
    perm = np.concatenate([
        np.concatenate([np.arange(o, o + w), FFN + np.arange(o, o + w)])
        for o, w in zip([0, 512, 1024], PAIR_W)
    ])

    w_gate_up = np.asarray(w_gate_up)
    w_down = np.asarray(w_down)
    ws_gate_up = np.asarray(ws_gate_up)
    ws_down = np.asarray(ws_down)

    def _q8(a):
        return np.clip(a * WSCALE, -15.5, 15.5).astype(F8E3)

    in_maps = []
    for c in range(N_CORES):
        # routed expert weights (fp8-e3m4, scaled), token batches (bf16)
        wgu = np.empty((EPC, 128, N_KH * 2816), dtype=F8E3)
        wdn = np.empty((EPC, 128, 2 * N_KF * 1024), dtype=F8E3)
        xts = np.zeros((EPC, 128, N_KH * C), dtype=BF16)
        wv = np.zeros((C, EPC), dtype=np.float32)
        for s in range(EPC):
            e = c * EPC + s
            wt = _q8(w_gate_up[e][perm].T)              # [H, 2816] interleaved
            off = 0
            for o, w in zip(PAIR_OFF, PAIR_W):
                blk = _sbufize(wt[:, o:o + 2 * w], N_KH)  # [128, 16*2w]
                wgu[s, :, off:off + blk.shape[1]] = blk
                off += blk.shape[1]
            wdT = _q8(w_down[e].T)                       # [F, H]
            for h in range(2):
                wdn[s, :, h * N_KF * 1024:(h + 1) * N_KF * 1024] = _sbufize(
                    wdT[:, h * 1024:(h + 1) * 1024], N_KF
                )
            toks = lists[e]
            n = len(toks)
            if n:
                xte = np.zeros((HIDDEN, C), dtype=np.float32)
                xte[:, :n] = xT[:, toks]
                xts[s] = _sbufize(xte.astype(BF16), N_KH)
                # per-token routing weights in expert order; the 1/WSCALE^2
                # undoes the u-side and down-side weight scales
                wcol = np.zeros(C, dtype=np.float32)
                for i in range(TOP_K):
                    sel = ids[:, i] == e
                    wcol[pos[sel, i]] = tw[sel, i]
                wv[:, s] = wcol / (WSCALE * WSCALE)
        # shared expert slice (tensor-parallel on intermediate dim)
        g_sl = ws_gate_up[c * SFS:(c + 1) * SFS]            # [352, H]
        u_sl = ws_gate_up[SHARED_FFN + c * SFS: SHARED_FFN + (c + 1) * SFS]
        wsgu = _sbufize(
            np.concatenate([g_sl, u_sl], axis=0).T.astype(BF16), N_KH
        )  # [128, 16*704]
        wsdT = ws_down[:, c * SFS:(c + 1) * SFS].T.astype(BF16)  # [352, H]
        wsd_pad = np.zeros((384, HIDDEN), dtype=BF16)
        wsd_pad[:SFS] = wsdT
        wsd = _sbufize(wsd_pad, 3)                          # [128, 3*2048]
        xsh = _sbufize(xT.astype(BF16), N_KH)               # [128, 16*256]
        in_maps.append({
            "xt": xts, "wgu": wgu, "wdn": wdn, "wv": wv,
            "xsh": xsh, "wsgu": wsgu, "wsd": wsd,
        })

    if C not in _PROGRAM_CACHE:
        _PROGRAM_CACHE[C] = _build_program(C)
    nc = _PROGRAM_CACHE[C]

    from concourse.bass_utils import run_bass_kernel_spmd
    res = run_bass_kernel_spmd(
        nc, in_maps, list(range(N_CORES)),
        trace=bool(os.environ.get("MOE_KERNEL_TRACE")),
    )
    LAST_RESULTS = res

    # ---- combine: gather-sum of weighted routed rows + shared partials ----
    y_all = np.stack([r["yrt"].astype(np.float32) for r in res.results])       # [8, EPC, C, H]
    y_flat = y_all.reshape(N_EXPERTS * C, HIDDEN)
    G = ids * C + pos                                       # [T, 6]
    routed = y_flat[G].sum(axis=1)
    shared = np.sum([r["ysh"].astype(np.float32) for r in res.results], axis=0)
    out = routed + shared
    return out.reshape(1, T, HIDDEN).astype(np.float32)



# revision 36
# speedup vs baseline: 1.0763x; 1.0139x over previous
"""DeepSeek-V2-Lite MoE layer on 8 Trainium2 NeuronCores.

Strategy (expert-parallel, per the sharding hint):
  - Host computes the gate (256x64 matmul + softmax + top-6) in fp32 numpy --
    this is the token dispatch decision, which necessarily lives on the host
    since the host builds the per-core input shards ("all-to-all" realized as
    host-side gather/scatter under the full-IO contract).
  - Each core owns 8 routed experts (weights sharded on the expert axis) and
    a 1/8 slice of the shared expert intermediate dim (tensor-parallel).
  - Tokens routed to each expert are gathered host-side into a fixed-capacity
    [C] batch (C = max expert load rounded up; uniform so the single SPMD
    program is identical across cores).
  - On device, all matmuls are token-stationary: the (small) token batch is
    the stationary PE operand, the expert weights stream through as the
    moving operand, so PE time ~ weight-columns/2.4GHz and the kernel is
    bound by the irreducible weight DMA (~138 MB/core in bf16).
  - Device applies the per-token routing weight; host combine is a pure
    gather-sum plus the 8-way shared-expert partial sum.
"""

import os
import numpy as np
import ml_dtypes

BF16 = ml_dtypes.bfloat16
F8E3 = ml_dtypes.float8_e3m4   # TRN FP8_EXP3: 4 mantissa bits, max 15.5

HIDDEN = 2048
FFN = 1408
N_EXPERTS = 64
TOP_K = 6
SHARED_FFN = 2816          # 2 shared experts * FFN
T = 256
N_CORES = 8
EPC = N_EXPERTS // N_CORES  # experts per core = 8
SFS = SHARED_FFN // N_CORES  # shared-FFN slice per core = 352

# routed expert weights are shipped in fp8-e3m4 scaled by WSCALE (so the
# +/-0.1-ish gaussian weights land in e3m4's normal range); the gate-side
# 1/WSCALE is undone inside the silu activation, the remaining WSCALE^2
# factor is folded into the per-token combine weights.
WSCALE = 64.0

# gate/up column interleave: stream order [g0|u0|g1|u1|g2|u2], pair widths
PAIR_W = [512, 512, 384]
PAIR_OFF = [0, 1024, 2048]          # start col of each pair block (2*w wide)
N_KH = HIDDEN // 128                # 16 K-chunks over hidden
N_KF = FFN // 128                   # 11 K-chunks over FFN

_PROGRAM_CACHE = {}
LAST_RESULTS = None


def _route(x, gate_w):
    """fp32 softmax top-k routing, matching jax.lax.top_k tie-breaking
    (stable sort -> lowest index wins ties)."""
    logits = x @ gate_w.T                      # [T, E] fp32
    m = logits.max(axis=-1, keepdims=True)
    e = np.exp(logits - m)
    scores = e / e.sum(axis=-1, keepdims=True)
    ids = np.argsort(-scores, axis=-1, kind="stable")[:, :TOP_K]
    w = np.take_along_axis(scores, ids, axis=-1)
    w = w / (w.sum(axis=-1, keepdims=True) + 1e-20)
    return ids, w.astype(np.float32)


def _build_program(C):
    import concourse.bass as bass
    import concourse.bacc as bacc
    import concourse.mybir as mybir
    import concourse.tile as tile
    from concourse.masks import make_identity
    from contextlib import ExitStack

    f32 = mybir.dt.float32
    bf16 = mybir.dt.bfloat16
    SILU = mybir.ActivationFunctionType.Silu
    COPY = mybir.ActivationFunctionType.Copy

    # Bacc (not plain Bass): its compile pipeline splits multi-wait
    # instructions into the 1-wait-per-instruction form TRN2 requires.
    nc = bacc.Bacc(None)

    f8 = mybir.dt.float8e3

    # DRAM layouts are host-prepped into final SBUF layouts so every weight
    # DMA is contiguous per partition row.
    W_GU = N_KH * 2816
    W_DN = 2 * N_KF * 1024
    d_xt = nc.dram_tensor("xt", [EPC, 128, N_KH * C], bf16, kind="ExternalInput")
    d_wgu = nc.dram_tensor("wgu", [EPC, 128, W_GU], f8, kind="ExternalInput")
    d_wdn = nc.dram_tensor("wdn", [EPC, 128, W_DN], f8, kind="ExternalInput")
    d_wv = nc.dram_tensor("wv", [C, EPC], f32, kind="ExternalInput")
    d_xsh = nc.dram_tensor("xsh", [128, N_KH * 256], bf16, kind="ExternalInput")
    d_wsgu = nc.dram_tensor("wsgu", [128, N_KH * 2 * SFS], bf16, kind="ExternalInput")
    d_wsd = nc.dram_tensor("wsd", [128, 3 * 2048], bf16, kind="ExternalInput")
    d_yrt = nc.dram_tensor("yrt", [EPC, C, HIDDEN], bf16, kind="ExternalOutput")
    d_ysh = nc.dram_tensor("ysh", [T, HIDDEN], bf16, kind="ExternalOutput")

    with tile.TileContext(nc) as tc, ExitStack() as ctx:
        p_const = ctx.enter_context(tc.tile_pool(name="const", bufs=1))
        p_wgu = ctx.enter_context(tc.tile_pool(name="wgu", bufs=3))
        p_wdn = ctx.enter_context(tc.tile_pool(name="wdn", bufs=2))
        p_wsh = ctx.enter_context(tc.tile_pool(name="wsh", bufs=1))
        p_xt = ctx.enter_context(tc.tile_pool(name="xt", bufs=2))
        p_act = ctx.enter_context(tc.tile_pool(name="act", bufs=2))
        p_gs = ctx.enter_context(tc.tile_pool(name="gs", bufs=2))
        p_actT = ctx.enter_context(tc.tile_pool(name="actT", bufs=2))
        p_out = ctx.enter_context(tc.tile_pool(name="out", bufs=2))
        # gate/up and down projections share one PSUM ring (same tag) so the
        # down matmuls never wait on a drain of their own dedicated buffer
        ps_gu = ctx.enter_context(tc.tile_pool(name="ps_gu", bufs=2, space="PSUM"))
        ps_t = ctx.enter_context(tc.tile_pool(name="ps_t", bufs=3, space="PSUM"))

        ident = p_const.tile([128, 128], bf16)
        make_identity(nc, ident[:])
        wv_t = p_const.tile([C, EPC], f32)
        nc.sync.dma_start(out=wv_t[:], in_=d_wv[:])

        state = {}

        def issue_wd(s):
            """down-weight DMA for slot s; issued one gu-block before use so
            the queue stays deadline-ordered (gu pairs are more urgent)"""
            wds = []
            for h in range(2):
                wd = p_wdn.tile([128, N_KF * 1024], f8, tag="wdn")
                nc.sync.dma_start(
                    out=wd[:],
                    in_=d_wdn[s, :, h * N_KF * 1024:(h + 1) * N_KF * 1024],
                )
                wds.append(wd)
            state[s] = state[s] + (wds,)

        def routed_gu(s, mid=None):
            """gate+up matmuls and silu*u for expert slot s; mid() is emitted
            between pair 1 and pair 2 (the previous expert's transposes run
            there, so their scalar-engine copies beat silu-p2 in the queue)"""
            xt = p_xt.tile([128, N_KH * C], bf16, tag="xt")
            nc.sync.dma_start(out=xt[:], in_=d_xt[s])
            act = p_act.tile([C, FFN], bf16, tag="act")
            # ---- gate+up, pair-major over 3 (g,u) column pairs ----
            for pr in range(3):
                w = PAIR_W[pr]
                if s == 0 and pr == 0:
                    # first weights: one tile per 4 k-chunks so the PE starts
                    # as soon as the first 512KB lands (tile-granular
                    # dependency tracking would otherwise wait for all 2MB;
                    # more splits would pay ~0.6us sync-engine trigger each)
                    wgs = []
                    for q in range(4):
                        wq = p_wgu.tile([128, 4 * 2 * w], f8, tag=f"wgu0_{q}",
                                        bufs=1)
                        nc.sync.dma_start(
                            out=wq[:],
                            in_=d_wgu[s, :, q * 8 * w:(q + 1) * 8 * w],
                        )
                        wgs.append(wq)
                    wgof = lambda k: (wgs[k // 4], (k % 4) * 2 * w)
                else:
                    wg = p_wgu.tile([128, N_KH * 2 * w], f8, tag="wgu")
                    nc.sync.dma_start(
                        out=wg[:],
                        in_=d_wgu[s, :, N_KH * PAIR_OFF[pr]:
                                 N_KH * (PAIR_OFF[pr] + 2 * w)],
                    )
                    wgof = lambda k: (wg, k * 2 * w)
                if pr == 2 and mid is not None:
                    mid()
                pg = ps_gu.tile([C, 1024], mybir.dt.float32, tag="pg")
                for k in range(N_KH):
                    lhs = xt[:, k * C:(k + 1) * C]
                    wt, o = wgof(k)
                    # matmul output is capped at one PSUM bank (512 fp32)
                    nc.tensor.matmul(
                        pg[:, 0:w], lhs, wt[:, o:o + w],
                        start=(k == 0), stop=(k == N_KH - 1),
                    )
                    nc.tensor.matmul(
                        pg[:, 512:512 + w], lhs, wt[:, o + w:o + 2 * w],
                        start=(k == 0), stop=(k == N_KH - 1),
                    )
                gs = p_gs.tile([C, 512], mybir.dt.float32, tag="gs")
                # silu(G~ / WSCALE) undoes the gate-side weight scale exactly
                nc.scalar.activation(gs[:, :w], pg[:, :w], SILU, scale=1.0 / WSCALE)
                nc.vector.tensor_mul(
                    act[:, pr * 512: pr * 512 + w], gs[:, :w], pg[:, 512:512 + w]
                )
            state[s] = (act,)

        def routed_mid(s):
            """transpose act (tokens->free) of slot s for its down matmul"""
            act = state[s][0]
            actT = p_actT.tile([128, N_KF * C], bf16, tag="actT")
            TG = (N_KF + 2) // 3
            j = 0
            while j < N_KF:
                g = min(TG, N_KF - j)
                pt = ps_t.tile([128, 512], bf16, tag="pt")
                for i in range(g):
                    nc.tensor.transpose(
                        pt[:, i * C:(i + 1) * C],
                        act[:, (j + i) * 128:(j + i + 1) * 128], ident[:C, :C]
                    )
                # drain on the (nearly idle) scalar engine: the DVE is blocked
                # behind the next expert's silu*u muls at exactly this moment
                nc.scalar.activation(actT[:, j * C:(j + g) * C], pt[:, :g * C],
                                     COPY)
                j += g
            state[s] = state[s] + (actT,)

        def routed_down(s):
            """down projection + drain for expert slot s"""
            act, actT, wds = state.pop(s)
            out_sb = p_out.tile([C, HIDDEN], bf16, tag="out")
            for h in range(2):
                py = ps_gu.tile([C, 1024], mybir.dt.float32, tag="pg")
                for k in range(N_KF):
                    for n in range(2):
                        nc.tensor.matmul(
                            py[:, n * 512:(n + 1) * 512],
                            actT[:, k * C:(k + 1) * C],
                            wds[h][:, k * 1024 + n * 512: k * 1024 + (n + 1) * 512],
                            start=(k == 0), stop=(k == N_KF - 1),
                        )
                # combine weight folded into the drain; split across the DVE
                # and scalar engines so the two halves drain in parallel
                nc.vector.tensor_scalar_mul(
                    out_sb[:, h * 1024:h * 1024 + 512], py[:, 0:512],
                    wv_t[:, s:s + 1]
                )
                nc.scalar.activation(
                    out_sb[:, h * 1024 + 512:(h + 1) * 1024], py[:, 512:1024],
                    COPY, scale=wv_t[:, s:s + 1]
                )
                # per-half output DMA: h0's transfer hides under h1's matmuls
                nc.sync.dma_start(out=d_yrt[s, :, h * 1024:(h + 1) * 1024],
                                  in_=out_sb[:, h * 1024:(h + 1) * 1024])

        def shared_expert():
            xsh = p_xt.tile([128, N_KH * 256], bf16, tag="xsh")
            nc.sync.dma_start(out=xsh[:], in_=d_xsh[:])
            wsg = p_wsh.tile([128, N_KH * 2 * SFS], bf16, tag="wsg")
            nc.sync.dma_start(out=wsg[:], in_=d_wsgu[:])
            wsd = p_wsh.tile([128, 3 * 2048], bf16, tag="wsd")
            nc.sync.dma_start(out=wsd[:], in_=d_wsd[:])
            # both token groups' gate/up first so group 1's matmuls hide the
            # silu->mul->transpose chain of group 0
            acts = []
            for g in range(2):  # two groups of 128 tokens
                pg = ps_gu.tile([128, 1024], mybir.dt.float32, tag="pg")
                for k in range(N_KH):
                    lhs = xsh[:, k * 256 + g * 128: k * 256 + g * 128 + 128]
                    nc.tensor.matmul(
                        pg[:, 0:SFS], lhs, wsg[:, k * 2 * SFS: k * 2 * SFS + SFS],
                        start=(k == 0), stop=(k == N_KH - 1),
                    )
                    nc.tensor.matmul(
                        pg[:, 512:512 + SFS],
                        lhs, wsg[:, k * 2 * SFS + SFS: (k + 1) * 2 * SFS],
                        start=(k == 0), stop=(k == N_KH - 1),
                    )
                gs = p_gs.tile([128, 512], mybir.dt.float32, tag="gs")
                nc.scalar.activation(gs[:, :SFS], pg[:, :SFS], SILU)
                act_sh = p_act.tile([128, SFS], bf16, tag="acts")
                nc.vector.tensor_mul(act_sh[:], gs[:, :SFS], pg[:, 512:512 + SFS])
                acts.append(act_sh)
            for g in range(2):
                act_sh = acts[g]
                actT_sh = p_actT.tile([128, 3 * 128], bf16, tag="actTs")
                # rows 96:128 of the last K-chunk pair with zero weight rows;
                # zero them so junk*0 can't produce NaN
                nc.vector.memset(actT_sh[:], 0.0)
                for j, wj in enumerate([128, 128, 96]):
                    pt = ps_t.tile([128, 128], bf16, tag="pt")
                    nc.tensor.transpose(
                        pt[:wj, :], act_sh[:, j * 128: j * 128 + wj], ident[:, :]
                    )
                    nc.scalar.activation(
                        actT_sh[:wj, j * 128:(j + 1) * 128], pt[:wj, :], COPY
                    )
                out_sh = p_out.tile([128, HIDDEN], bf16, tag="outs")
                for h in range(2):
                    py = ps_gu.tile([128, 1024], mybir.dt.float32, tag="pg")
                    for k in range(3):
                        for n in range(2):
                            nc.tensor.matmul(
                                py[:, n * 512:(n + 1) * 512],
                                actT_sh[:, k * 128:(k + 1) * 128],
                                wsd[:, k * 2048 + h * 1024 + n * 512:
                                    k * 2048 + h * 1024 + (n + 1) * 512],
                                start=(k == 0), stop=(k == 2),
                            )
                    nc.vector.tensor_copy(out_sh[:, h * 1024:(h + 1) * 1024], py[:])
                nc.sync.dma_start(out=d_ysh[g * 128:(g + 1) * 128, :], in_=out_sh[:])

        # one-expert software-pipeline skew: expert s-1's transposes slot in
        # between expert s's gu pairs 1 and 2 (so their scalar copies beat
        # silu-p2 in the scalar queue) and its down matmuls follow pair 2
        # immediately; DMA issue stays deadline-ordered (gu pairs first, then
        # the previous expert's down weights). The shared expert fills the
        # drain chain of the last slot and keeps the kernel tail small.
        routed_gu(0)
        for s in range(1, EPC):
            routed_gu(s, mid=lambda prev=s - 1: routed_mid(prev))
            issue_wd(s - 1)
            routed_down(s - 1)
        shared_expert()
        routed_mid(EPC - 1)
        issue_wd(EPC - 1)
        routed_down(EPC - 1)

    if not nc.is_finalized():
        nc.finalize()
    return nc


def _sbufize(a, kdim):
    """[K*128, N] -> [128, K*N] SBUF layout (K-chunks along free dim)."""
    K = a.shape[0] // 128
    return np.ascontiguousarray(
        a.reshape(K, 128, -1).transpose(1, 0, 2).reshape(128, -1)
    )


def kernel(hidden_states, gate_w, w_gate_up, w_down, ws_gate_up, ws_down):
    global LAST_RESULTS
    x = np.asarray(hidden_states, dtype=np.float32).reshape(T, HIDDEN)
    gate_w = np.asarray(gate_w, dtype=np.float32)

    ids, tw = _route(x, gate_w)

    # per-expert token lists + positions
    lists = [[] for _ in range(N_EXPERTS)]
    pos = np.zeros((T, TOP_K), dtype=np.int64)
    for t in range(T):
        for i in range(TOP_K):
            e = ids[t, i]
            pos[t, i] = len(lists[e])
            lists[e].append(t)
    maxload = max(len(l) for l in lists)
    C = max(32, -(-maxload // 16) * 16)
    assert C <= 128, f"expert overload {maxload}: splitting not implemented"

    xb = x.astype(BF16)
    xT = np.ascontiguousarray(x.T)  # fp32 [H, T]

    # column permutation interleaving gate/up rows into [g0|u0|g1|u1|g2|u2]
    perm = np.concatenate([
        np.concatenate([np.arange(o, o + w), FFN + np.arange(o, o + w)])
        for o, w in zip([0, 512, 1024], PAIR_W)
    ])

    w_gate_up = np.asarray(w_gate_up)
    w_down = np.asarray(w_down)
    ws_gate_up = np.asarray(ws_gate_up)
    ws_down = np.asarray(ws_down)

    def _q8(a):
        return np.clip(a * WSCALE, -15.5, 15.5).astype(F8E3)

    in_maps = []
    for c in range(N_CORES):
        # routed expert weights (fp8-e3m4, scaled), token batches (bf16)
        wgu = np.empty((EPC, 128, N_KH * 2816), dtype=F8E3)
        wdn = np.empty((EPC, 128, 2 * N_KF * 1024), dtype=F8E3)
        xts = np.zeros((EPC, 128, N_KH * C), dtype=BF16)
        wv = np.zeros((C, EPC), dtype=np.float32)
        for s in range(EPC):
            e = c * EPC + s
            wt = _q8(w_gate_up[e][perm].T)              # [H, 2816] interleaved
            off = 0
            for o, w in zip(PAIR_OFF, PAIR_W):
                blk = _sbufize(wt[:, o:o + 2 * w], N_KH)  # [128, 16*2w]
                wgu[s, :, off:off + blk.shape[1]] = blk
                off += blk.shape[1]
            wdT = _q8(w_down[e].T)                       # [F, H]
            for h in range(2):
                wdn[s, :, h * N_KF * 1024:(h + 1) * N_KF * 1024] = _sbufize(
                    wdT[:, h * 1024:(h + 1) * 1024], N_KF
                )
            toks = lists[e]
            n = len(toks)
            if n:
                xte = np.zeros((HIDDEN, C), dtype=np.float32)
                xte[:, :n] = xT[:, toks]
                xts[s] = _sbufize(xte.astype(BF16), N_KH)
                # per-token routing weights in expert order; the 1/WSCALE^2
                # undoes the u-side and down-side weight scales
                wcol = np.zeros(C, dtype=np.float32)
                for i in range(TOP_K):
                    sel = ids[:, i] == e
                    wcol[pos[sel, i]] = tw[sel, i]
                wv[:, s] = wcol / (WSCALE * WSCALE)
        # shared expert slice (tensor-parallel on intermediate dim)
        g_sl = ws_gate_up[c * SFS:(c + 1) * SFS]            # [352, H]
        u_sl = ws_gate_up[SHARED_FFN + c * SFS: SHARED_FFN + (c + 1) * SFS]
        wsgu = _sbufize(
            np.concatenate([g_sl, u_sl], axis=0).T.astype(BF16), N_KH
        )  # [128, 16*704]
        wsdT = ws_down[:, c * SFS:(c + 1) * SFS].T.astype(BF16)  # [352, H]
        wsd_pad = np.zeros((384, HIDDEN), dtype=BF16)
        wsd_pad[:SFS] = wsdT
        wsd = _sbufize(wsd_pad, 3)                          # [128, 3*2048]
        xsh = _sbufize(xT.astype(BF16), N_KH)               # [128, 16*256]
        in_maps.append({
            "xt": xts, "wgu": wgu, "wdn": wdn, "wv": wv,
            "xsh": xsh, "wsgu": wsgu, "wsd": wsd,
        })

    if C not in _PROGRAM_CACHE:
        _PROGRAM_CACHE[C] = _build_program(C)
    nc = _PROGRAM_CACHE[C]

    from concourse.bass_utils import run_bass_kernel_spmd
    res = run_bass_kernel_spmd(
        nc, in_maps, list(range(N_CORES)),
        trace=bool(os.environ.get("MOE_KERNEL_TRACE")),
    )
    LAST_RESULTS = res

    # ---- combine: gather-sum of weighted routed rows + shared partials ----
    y_all = np.stack([r["yrt"].astype(np.float32) for r in res.results])       # [8, EPC, C, H]
    y_flat = y_all.reshape(N_EXPERTS * C, HIDDEN)
    G = ids * C + pos                                       # [T, 6]
    routed = y_flat[G].sum(axis=1)
    shared = np.sum([r["ysh"].astype(np.float32) for r in res.results], axis=0)
    out = routed + shared
    return out.reshape(1, T, HIDDEN).astype(np.float32)

